# revision 40
# baseline (speedup 1.0000x reference)
"""Trainium2 Bass kernel for causal+padded multi-head attention.

Problem: B=2, N=2048, D=1024, H=16 heads (DK=64), fp32 I/O.
  out = softmax(mask(x Wq^T (x Wk^T)^T) / sqrt(DK)) (x Wv^T) Wout^T + b_out

Sharding (8 cores): core c handles batch b=c//4 and heads [4*(c%4), 4*(c%4)+4).
Each core computes a partial output [N, D] (its 4 heads' contribution through
the output projection); the host sums the 4 partials per batch and adds b_out.

On-device layout (per core):
  xT   [1024, 2048]  (host-pretransposed x[b])
  QT/KT stored transposed [dk, n] as head-pair tiles [128, 2048]
  V    stored natural as [128(keys), 16 blocks, 4 heads, 65] with a ones
       column appended (col 64) so P@V' also yields the softmax denominator.
  S^T  computed per (head-pair, q-tile 512, key-block 128) as [128, 2, 512]
       in PSUM: matmul(lhsT=KT slice [64,128], rhs=QT slice [64,512]).
       Causal masking = additive -30000 on PSUM (DVE); padding mask is a
       per-key bias fused into the exp; one exp(0.125*s + bias) on ScalarE
       writes P^T straight to SBUF as bf16.
  ctx'^T [65, 512] accumulated in PSUM over key blocks:
       matmul(lhsT=V' [128,65], rhs=P^T [128,512]); PV matmuls are emitted
       one unit behind their exps so the in-order PE never waits on ScalarE.
  Normalization: r = recip(rowsum) on the [1,512] denominator row via the
       fast approx DVE reciprocal, partition-broadcast to [64,512] (GpSimd),
       one DVE multiply; the whole chain is emitted at the START of the
       following unit so it lands early in the in-order DVE queue, and the
       output projection is emitted after that unit's S^T/PV riffle so the
       PE reaches it only after the chain has drained.
  Out projection: matmul(lhsT=ctxT [128,128], rhs=WoutT [128,512]) acc over
       the two head-pair chunks; PSUM -> SBUF staging copy on DVE.

ScalarE runs ONLY the exps (plus a warm-up activation at kernel start that
preloads the Exp table off the critical path); all copies live on DVE.

Phase B (projections) is restructured e-outer: per 128-row chunk e of xT,
all four Q/K accumulation tiles of a head pair advance one step, so the PE
starts as soon as the first xT chunk + W_Q land instead of after the whole
4.2 MB xT DMA. Q/K for one pair use 8 concurrent PSUM banks.

All matmul operands are bf16 (pre-rounded on host for the inputs; on-device
casts for intermediates); accumulation is fp32 in PSUM, and the softmax /
masking / normalization arithmetic is fp32.
"""

import math
import os

import numpy as np

B, N, D, H = 2, 2048, 1024, 16
DK = D // H  # 64
NCORES = 8
HEADS_PER_CORE = 4
QTILE = 512
KBLK = 128
NEG = -30000.0
NEGB = -3750.0  # pad bias applied after the 0.125 scale inside exp
SCALE = 1.0 / math.sqrt(float(DK))  # 0.125

# Set by run() when tracing is enabled (test.py reads this).
LAST_RESULTS = None


def _build_program(kb_max: int, jpad_min: int):
    import concourse.tile as tile
    from concourse import bacc, mybir

    F32 = mybir.dt.float32
    BF16 = mybir.dt.bfloat16
    EXP = mybir.ActivationFunctionType.Exp
    ADD = mybir.AluOpType.add

    nc = bacc.Bacc(None)

    xt_d = nc.dram_tensor("xt", [D, N], BF16, kind="ExternalInput")
    wq_d = nc.dram_tensor("wq", [D, 256], BF16, kind="ExternalInput")
    wk_d = nc.dram_tensor("wk", [D, 256], BF16, kind="ExternalInput")
    wv_d = nc.dram_tensor("wv", [D, 256], BF16, kind="ExternalInput")
    wout_d = nc.dram_tensor("wout", [256, D], BF16, kind="ExternalInput")
    padb_d = nc.dram_tensor("padbias", [128, 16], F32, kind="ExternalInput")
    trineg_d = nc.dram_tensor("trineg", [128, 896], F32, kind="ExternalInput")
    ones_d = nc.dram_tensor("ones65", [128, 64], BF16, kind="ExternalInput")
    out_d = nc.dram_tensor("out", [N, D], mybir.dt.float16, kind="ExternalOutput")

    NB = N // KBLK  # 16 key/row blocks
    NQT = N // QTILE  # 4 q tiles

    with tile.TileContext(nc) as tc:
        with (
            tc.tile_pool(name="w", bufs=1) as w_pool,
            tc.tile_pool(name="big", bufs=1) as big_pool,
            tc.tile_pool(name="work", bufs=2) as work_pool,
        ):
            # ---- load inputs (order = arrival order; xt right after Q/K
            # weights so phase B starts ~11us in; masks/wout arrive later,
            # ahead of their first use in phase C/D) ----
            xt_cm = tc.tile_pool(name="xt", bufs=8)
            xt_pool = xt_cm.__enter__()
            padb_t = w_pool.tile([128, 16], F32, tag="padb")
            trineg_t = w_pool.tile([128, 896], F32, tag="trineg")
            wq_t = w_pool.tile([128, 8, 256], BF16, tag="wq")
            wk_t = w_pool.tile([128, 8, 256], BF16, tag="wk")
            wv_t = w_pool.tile([128, 8, 256], BF16, tag="wv")
            wo_t = w_pool.tile([128, 2, D], BF16, tag="wo")
            # wq then xt[0] first: the very first B1 matmul (Q, e=0) can
            # start before wk and the remaining xt chunks land
            nc.sync.dma_start(wq_t[:], wq_d[:].rearrange("(e p) m -> p e m", p=128))
            xt = []
            t0 = xt_pool.tile([128, N], BF16, tag="xt", name="xt0")
            nc.sync.dma_start(t0[:], xt_d[0:128, :])
            xt.append(t0)
            nc.sync.dma_start(wk_t[:], wk_d[:].rearrange("(e p) m -> p e m", p=128))
            for e in range(1, 8):
                t = xt_pool.tile([128, N], BF16, tag="xt")
                nc.sync.dma_start(t[:], xt_d[e * 128:(e + 1) * 128, :])
                xt.append(t)
            nc.sync.dma_start(wv_t[:], wv_d[:].rearrange("(e p) m -> p e m", p=128))
            nc.sync.dma_start(trineg_t[:], trineg_d[:])
            nc.sync.dma_start(padb_t[:], padb_d[:])

            # V' tile: [keys 128, key-block 16, head 4, 65]; col 64 <- ones
            v4 = big_pool.tile([128, NB, 4, 65], BF16, tag="v4")
            nc.sync.dma_start(
                v4[:, :, :, 64:65],
                ones_d[:].rearrange("p (b h o) -> p b h o", h=4, o=1),
            )
            # wout is not needed until phase D
            nc.sync.dma_start(wo_t[:], wout_d[:].rearrange("(c p) m -> p c m", p=128))

            # Preload the Exp activation table off the critical path: the
            # first real exp otherwise eats a 1.3us ACT_TABLE_LOAD mid-kernel.
            warm_t = work_pool.tile([1, 2], F32, tag="warm", name="warm")
            nc.vector.memset(warm_t[:, 0:1], 1.0)
            nc.scalar.activation(warm_t[:, 1:2], warm_t[:, 0:1], EXP, scale=SCALE)

            qt_pair = [big_pool.tile([128, N], BF16, tag=f"qt{p}", name=f"qt{p}") for p in range(2)]
            kt_pair = [big_pool.tile([128, N], BF16, tag=f"kt{p}", name=f"kt{p}") for p in range(2)]
            ctx_pair = [big_pool.tile([128, N], BF16, tag=f"ctx{p}", name=f"ctx{p}") for p in range(2)]

            # ---- phase B: projections ----
            # e-outer so compute starts when xt[0] lands: per 128-row chunk e
            # all 8 Q/K accumulation tiles (4 q-tiles x {Q,K}) of one head
            # pair advance one step. 8 PSUM banks; weight per (e, Q/K) is
            # loaded once and reused across the 4 q-tile matmuls.
            psb_cm = tc.tile_pool(name="psb", bufs=1, space="PSUM")
            psb = psb_cm.__enter__()
            for pair in range(2):
                ps_q = [
                    psb.tile([128, 512], F32, tag=f"bq{i}", name=f"bq{i}")
                    for i in range(NQT)
                ]
                ps_k = [
                    psb.tile([128, 512], F32, tag=f"bk{i}", name=f"bk{i}")
                    for i in range(NQT)
                ]
                for e in range(8):
                    for w_t, ps in ((wq_t, ps_q), (wk_t, ps_k)):
                        for nq in range(NQT):
                            nc.tensor.matmul(
                                ps[nq][:],
                                w_t[:, e, pair * 128:(pair + 1) * 128],
                                xt[e][:, nq * 512:(nq + 1) * 512],
                                start=(e == 0),
                                stop=(e == 7),
                            )
                # copies split across Scalar (idle in phase B) and DVE so the
                # 8-deep copy chain doesn't serialize on one engine — the
                # next phase's first PSUM writers WAR-wait on these reads
                for nq in range(NQT):
                    nc.scalar.copy(
                        qt_pair[pair][:, nq * 512:(nq + 1) * 512], ps_q[nq][:]
                    )
                    nc.vector.tensor_copy(
                        kt_pair[pair][:, nq * 512:(nq + 1) * 512], ps_k[nq][:]
                    )
            psb_cm.__exit__(None, None, None)

            ps_cm = tc.tile_pool(name="ps_main", bufs=3, space="PSUM")
            ps_main = ps_cm.__enter__()
            psc_cm = tc.tile_pool(name="ps_ctx", bufs=1, space="PSUM")
            ps_ctx = psc_cm.__enter__()
            pt_cm = tc.tile_pool(name="pt", bufs=26)
            pt_pool = pt_cm.__enter__()

            # V natural: [n-block, 4*64] = xT-chunk^T @ WvT-chunk. Emitted
            # lazily inside phase C (blocks land at q-tile starts, exactly
            # where the PE otherwise waits for the exp stream); blocks
            # >= kb_max are fully masked and never computed at all.
            v_next = [0]

            def ensure_v(k):
                while v_next[0] < min(k, kb_max):
                    nb = v_next[0]
                    v_next[0] += 1
                    vps = ps_main.tile(
                        [128, 2, 512], F32, tag="blk", name="vps"
                    )[:, 0, 0:256]
                    for e in range(8):
                        nc.tensor.matmul(
                            vps[:],
                            xt[e][:, nb * 128:(nb + 1) * 128],
                            wv_t[:, e, :],
                            start=(e == 0),
                            stop=(e == 7),
                        )
                    # first blocks copy on Scalar (idle until the first exps)
                    veng = nc.scalar.copy if nb < 4 else nc.vector.tensor_copy
                    veng(
                        v4[:, nb, :, 0:64],
                        vps[:].rearrange("p (h d) -> p h d", h=4),
                    )

            # ---- phase C: attention, head pairs interleaved ----
            # A unit is (head-pair, q-tile). The two heads' S^T matmuls sit
            # at base partitions 0 / 64. PV matmuls run one unit behind their
            # exps so the in-order PE never drains waiting on ScalarE.
            def emit_normalize(pair, hh, qt, ctx_ps):
                hp = slice(64 * hh, 64 * hh + 64)
                # Denominator row copied to partition 0 FIRST (the reciprocal
                # chain is the long pole; also the custom-DVE approx
                # reciprocal mishandles source APs at a nonzero partition
                # offset, so it must read a partition-0 tile).
                dcp = work_pool.tile([1, 512], F32, tag="dcp", name="dcp")
                nc.vector.tensor_copy(dcp[:], ctx_ps[64:65, :])
                rrec = work_pool.tile([1, 512], F32, tag="rrec", name="rrec")
                nc.vector.reciprocal_approx_fast(rrec[:], dcp[:])
                # ctx rows staged to SBUF promptly so the PSUM bank frees for
                # the next unit's PV long before the chain completes
                craw = work_pool.tile([64, 512], F32, tag="craw", name="craw")
                nc.vector.tensor_copy(craw[:], ctx_ps[0:64, :])
                rbr = work_pool.tile([64, 512], F32, tag="rbr", name="rbr")
                nc.gpsimd.partition_broadcast(rbr[:], rrec[:])
                # multiply on GpSimd (SBUF-only operands): keeps the qt-
                # boundary DVE queue short so out-proj weight loads drain
                nc.gpsimd.tensor_mul(
                    ctx_pair[pair][hp, qt * 512:(qt + 1) * 512],
                    craw[:],
                    rbr[:],
                )

            def emit_st_exp(pair, qt, nchunks, prev):
                """S^T + mask + exp for both heads, with the previous unit's
                PV matmuls riffled in (they are long-ready and fill the PE
                slots where S^T would stall on the exp pipeline). Returns
                PV descriptors."""
                if prev is None:
                    ppv = []
                else:
                    ppair, pqt, pn, ppv, pctx2 = prev

                def rif(k):
                    # emit previous-unit PV chunks up to index k
                    while ppv and ppv[0][0] <= k:
                        jj, ptt, poff = ppv.pop(0)
                        for hh in range(2):
                            nc.tensor.matmul(
                                pctx2[hh][:, poff:],
                                v4[:, jj, 2 * ppair + hh, :],
                                ptt[:, hh, poff:],
                                start=(jj == 0),
                                stop=(jj == pn - 1),
                                skip_group_check=True,
                            )

                pv = []
                for j in range(nchunks):
                    rif(j)
                    d = j - 4 * qt
                    # exact-causal column trim (keep matmul N >= 256)
                    off = 128 * d if d >= 1 else 0
                    st_ps = ps_main.tile([128, 2, 512], F32, tag="blk", name="blk")
                    for hh in range(2):
                        hp = slice(64 * hh, 64 * hh + 64)
                        nc.tensor.matmul(
                            st_ps[:, hh, off:],
                            kt_pair[pair][hp, j * 128:(j + 1) * 128],
                            qt_pair[pair][hp, qt * 512 + off:(qt + 1) * 512],
                            start=True,
                            stop=True,
                        )
                    if d >= 0:
                        # causal add -30000; with off = 128*d the masked
                        # triangle lies entirely in cols [off, off+128)
                        u0 = 384 - 128 * d + off
                        w = min(128, 512 - off)
                        meng = (
                            nc.gpsimd
                            if os.environ.get("KERNEL_MASK_GPSIMD", "0") == "1"
                            else nc.vector
                        )
                        for hh in range(2):
                            meng.tensor_tensor(
                                st_ps[:, hh, off:off + w],
                                st_ps[:, hh, off:off + w],
                                trineg_t[:, u0:u0 + w],
                                ADD,
                            )
                    pt_t = pt_pool.tile([128, 2, 512], BF16, tag="pt")
                    kw = {}
                    if j >= jpad_min:  # per-key pad bias (same for both heads)
                        kw["bias"] = padb_t[:, j:j + 1]
                    nc.scalar.activation(
                        pt_t[:, :, off:], st_ps[:, :, off:], EXP, scale=SCALE, **kw
                    )
                    pv.append((j, pt_t, off))
                rif(10 ** 9)
                return pv

            def emit_pv(pair, qt, nchunks, pv, ctx2):
                for j, pt_t, off in pv:
                    for hh in range(2):
                        nc.tensor.matmul(
                            ctx2[hh][:, off:],
                            v4[:, j, 2 * pair + hh, :],
                            pt_t[:, hh, off:],
                            start=(j == 0),
                            stop=(j == nchunks - 1),
                            skip_group_check=True,
                        )

            units = [
                (pair, qt, min(4 * qt + 4, kb_max))
                for qt in range(NQT)
                for pair in range(2)
            ]
            done_norms = {q: 0 for q in range(NQT)}
            outproj_pending = []

            def emit_outproj(q):
                # output projection for the 4 n-blocks of q-tile q; one blk
                # tile per nb (fc halves in its two banks), one fp16 staging
                # copy, one DMA.
                F16 = mybir.dt.float16
                for nb in range(4 * q, 4 * q + 4):
                    ps = ps_main.tile([128, 2, 512], F32, tag="blk", name="blk")
                    for fc in range(2):
                        for pr2 in range(2):
                            nc.tensor.matmul(
                                ps[:, fc, :],
                                ctx_pair[pr2][:, nb * 128:(nb + 1) * 128],
                                wo_t[:, pr2, fc * 512:(fc + 1) * 512],
                                start=(pr2 == 0),
                                stop=(pr2 == 1),
                            )
                    # staging casts split across Scalar (fc0) and DVE (fc1)
                    # so neither queue eats the full 1.2us, and the two DMA
                    # halves start as soon as their own cast lands
                    osb = work_pool.tile([128, 2, 512], F16, tag="osb", name="osb")
                    nc.scalar.copy(osb[:, 0, :], ps[:, 0, :])
                    nc.vector.tensor_copy(osb[:, 1, :], ps[:, 1, :])
                    for fc in range(2):
                        nc.sync.dma_start(
                            out_d[nb * 128:(nb + 1) * 128,
                                  fc * 512:(fc + 1) * 512],
                            osb[:, fc, :],
                        )

            def pop_norm():
                npair, nqt, nctx2 = norm_q.pop(0)
                for hh in range(2):
                    emit_normalize(npair, hh, nqt, nctx2[hh])
                done_norms[nqt] += 1
                if done_norms[nqt] == 2:
                    outproj_pending.append(nqt)

            prev_pv = None  # (pair, qt, nchunks, pv_descs, ctx2)
            norm_q = []  # normalize one unit behind the PV
            for pair, qt, nchunks in units:
                # Emit the pending normalize chain FIRST so its DVE/GpSimd
                # work sits ahead of this unit's mask adds in the in-order
                # queues (its data deps completed a unit ago).
                if len(norm_q) > 1:
                    pop_norm()
                # V blocks this unit's PV will need (riffled next iteration)
                ensure_v(nchunks)
                pv = emit_st_exp(pair, qt, nchunks, prev_pv)
                if prev_pv is not None:
                    ppair, pqt, pn, ppv, pctx2 = prev_pv
                    norm_q.append((ppair, pqt, pctx2))
                # Out-projections go AFTER the riffle: by the time the PE
                # reaches them the normalize chain has drained.
                while outproj_pending:
                    emit_outproj(outproj_pending.pop(0))
                ctx2 = [
                    ps_ctx.tile([65, 512], F32, tag=f"ctx{hh}", name=f"ctx{hh}")
                    for hh in range(2)
                ]
                prev_pv = (pair, qt, nchunks, pv, ctx2)
            # Epilogue: the second-to-last unit's normalize chain (its PV
            # finished during the last riffle) is emitted BEFORE the last
            # unit's PV matmuls so the chain drains while the PE works.
            ppair, pqt, pn, ppv, pctx2 = prev_pv
            if norm_q:
                pop_norm()
            emit_pv(ppair, pqt, pn, ppv, pctx2)
            norm_q.append((ppair, pqt, pctx2))
            while norm_q:
                pop_norm()
            while outproj_pending:
                emit_outproj(outproj_pending.pop(0))

            pt_cm.__exit__(None, None, None)
            psc_cm.__exit__(None, None, None)
            ps_cm.__exit__(None, None, None)
            xt_cm.__exit__(None, None, None)

    nc.compile()
    return nc


_PROGRAM_CACHE = {}


def kernel(x, attention_mask, W_Q, W_K, W_V, W_out, b_out):
    global LAST_RESULTS
    from concourse.bass_utils import run_bass_kernel_spmd

    x = np.ascontiguousarray(x, dtype=np.float32)
    attention_mask = np.asarray(attention_mask)
    lengths = attention_mask.astype(np.int64).sum(axis=1)
    kb_max = int(math.ceil(lengths.max() / KBLK))
    jpad_min = int(lengths.min() // KBLK)

    key = (kb_max, jpad_min)
    if key not in _PROGRAM_CACHE:
        _PROGRAM_CACHE[key] = _build_program(kb_max, jpad_min)
    nc = _PROGRAM_CACHE[key]

    # host-side input prep (matmul operands pre-cast to bf16)
    import ml_dtypes
    BF = ml_dtypes.bfloat16
    xT = [np.ascontiguousarray(x[b].T.astype(BF)) for b in range(B)]
    wqT = np.ascontiguousarray(np.asarray(W_Q, dtype=np.float32).T.astype(BF))
    wkT = np.ascontiguousarray(np.asarray(W_K, dtype=np.float32).T.astype(BF))
    wvT = np.ascontiguousarray(np.asarray(W_V, dtype=np.float32).T.astype(BF))
    woT = np.ascontiguousarray(np.asarray(W_out, dtype=np.float32).T.astype(BF))
    # padbias[p, j] = 0 if key j*128+p is real else -3750
    padb = [
        np.ascontiguousarray(
            np.where(attention_mask[b].reshape(16, 128).T != 0, 0.0, NEGB)
        ).astype(np.float32)
        for b in range(B)
    ]
    # trineg[p, u] = NEG if u < p + 384 else 0; slice [384-128d : 896-128d]
    # gives the causal additive mask for a diagonal block with offset 128d.
    pp = np.arange(128)[:, None]
    uu = np.arange(896)[None, :]
    trineg = np.where(uu < pp + 384, NEG, 0.0).astype(np.float32)
    ones65 = np.ones((128, 64), dtype=BF)

    in_maps = []
    for c in range(NCORES):
        b, g = divmod(c, 4)
        sl = slice(g * 256, (g + 1) * 256)
        in_maps.append(
            {
                "xt": xT[b],
                "wq": np.ascontiguousarray(wqT[:, sl]),
                "wk": np.ascontiguousarray(wkT[:, sl]),
                "wv": np.ascontiguousarray(wvT[:, sl]),
                "wout": np.ascontiguousarray(woT[sl, :]),
                "padbias": padb[b],
                "trineg": trineg,
                "ones65": ones65,
            }
        )

    trace = bool(int(os.environ.get("KERNEL_TRACE", "0")))
    ncores_run = int(os.environ.get("KERNEL_NCORES", str(NCORES)))
    res = run_bass_kernel_spmd(
        nc,
        in_maps[:ncores_run],
        core_ids=list(range(ncores_run)),
        trace=trace,
        trace_cores=list(range(ncores_run)) if trace else None,
    )
    LAST_RESULTS = res

    out = np.zeros((B, N, D), dtype=np.float32)
    for c in range(len(res.results)):
        out[c // 4] += res.results[c]["out"].astype(np.float32)
    out += np.asarray(b_out, dtype=np.float32)[None, None, :]
    return out


# revision 41
# speedup vs baseline: 2.2022x; 2.2022x over previous
"""Trainium2 Bass kernel for causal+padded multi-head attention.

Problem: B=2, N=2048, D=1024, H=16 heads (DK=64), fp32 I/O.
  out = softmax(mask(x Wq^T (x Wk^T)^T) / sqrt(DK)) (x Wv^T) Wout^T + b_out

Sharding (8 cores): core c handles batch b=c//4 and heads [4*(c%4), 4*(c%4)+4).
Each core computes a partial output [N, D] (its 4 heads' contribution through
the output projection); the host sums the 4 partials per batch and adds b_out.

On-device layout (per core):
  xT   [1024, 2048]  (host-pretransposed x[b])
  QT/KT stored transposed [dk, n] as head-pair tiles [128, 2048]
  V    stored natural as [128(keys), 16 blocks, 4 heads, 65] with a ones
       column appended (col 64) so P@V' also yields the softmax denominator.
  S^T  computed per (head-pair, q-tile 512, key-block 128) as [128, 2, 512]
       in PSUM: matmul(lhsT=KT slice [64,128], rhs=QT slice [64,512]).
       Causal masking = additive -30000 on PSUM (DVE); padding mask is a
       per-key bias fused into the exp; one exp(0.125*s + bias) on ScalarE
       writes P^T straight to SBUF as bf16.
  ctx'^T [65, 512] accumulated in PSUM over key blocks:
       matmul(lhsT=V' [128,65], rhs=P^T [128,512]); PV matmuls are emitted
       one unit behind their exps so the in-order PE never waits on ScalarE.
  Normalization: r = recip(rowsum) on the [1,512] denominator row via the
       fast approx DVE reciprocal, partition-broadcast to [64,512] (GpSimd),
       one DVE multiply; the whole chain is emitted at the START of the
       following unit so it lands early in the in-order DVE queue, and the
       output projection is emitted after that unit's S^T/PV riffle so the
       PE reaches it only after the chain has drained.
  Out projection: matmul(lhsT=ctxT [128,128], rhs=WoutT [128,512]) acc over
       the two head-pair chunks; PSUM -> SBUF staging copy on DVE.

ScalarE runs ONLY the exps (plus a warm-up activation at kernel start that
preloads the Exp table off the critical path); all copies live on DVE.

Phase B (projections) is restructured e-outer: per 128-row chunk e of xT,
all four Q/K accumulation tiles of a head pair advance one step, so the PE
starts as soon as the first xT chunk + W_Q land instead of after the whole
4.2 MB xT DMA. Q/K for one pair use 8 concurrent PSUM banks.

All matmul operands are bf16 (pre-rounded on host for the inputs; on-device
casts for intermediates); accumulation is fp32 in PSUM, and the softmax /
masking / normalization arithmetic is fp32.
"""

import math
import os

import numpy as np

B, N, D, H = 2, 2048, 1024, 16
DK = D // H  # 64
NCORES = 8
HEADS_PER_CORE = 4
QTILE = 512
KBLK = 128
NEG = -30000.0
NEGB = -3750.0  # pad bias applied after the 0.125 scale inside exp
SCALE = 1.0 / math.sqrt(float(DK))  # 0.125

# Set by run() when tracing is enabled (test.py reads this).
LAST_RESULTS = None


def _build_program(kb_max: int, jpad_min: int):
    import concourse.tile as tile
    from concourse import bacc, mybir

    F32 = mybir.dt.float32
    BF16 = mybir.dt.bfloat16
    EXP = mybir.ActivationFunctionType.Exp
    ADD = mybir.AluOpType.add

    nc = bacc.Bacc(None)

    xt_d = nc.dram_tensor("xt", [D, N], BF16, kind="ExternalInput")
    wq_d = nc.dram_tensor("wq", [D, 256], BF16, kind="ExternalInput")
    wk_d = nc.dram_tensor("wk", [D, 256], BF16, kind="ExternalInput")
    wv_d = nc.dram_tensor("wv", [D, 256], BF16, kind="ExternalInput")
    wout_d = nc.dram_tensor("wout", [256, D], BF16, kind="ExternalInput")
    padb_d = nc.dram_tensor("padbias", [128, 16], F32, kind="ExternalInput")
    trineg_d = nc.dram_tensor("trineg", [128, 896], F32, kind="ExternalInput")
    ones_d = nc.dram_tensor("ones65", [128, 64], BF16, kind="ExternalInput")
    out_d = nc.dram_tensor("out", [N, D], mybir.dt.float16, kind="ExternalOutput")

    NB = N // KBLK  # 16 key/row blocks
    NQT = N // QTILE  # 4 q tiles

    with tile.TileContext(nc) as tc:
        with (
            tc.tile_pool(name="w", bufs=1) as w_pool,
            tc.tile_pool(name="big", bufs=1) as big_pool,
            tc.tile_pool(name="work", bufs=2) as work_pool,
        ):
            # ---- load inputs (order = arrival order; xt right after Q/K
            # weights so phase B starts ~11us in; masks/wout arrive later,
            # ahead of their first use in phase C/D) ----
            xt_cm = tc.tile_pool(name="xt", bufs=8)
            xt_pool = xt_cm.__enter__()
            padb_t = w_pool.tile([128, 16], F32, tag="padb")
            trineg_t = w_pool.tile([128, 896], F32, tag="trineg")
            wq_t = w_pool.tile([128, 8, 256], BF16, tag="wq")
            wk_t = w_pool.tile([128, 8, 256], BF16, tag="wk")
            wv_t = w_pool.tile([128, 8, 256], BF16, tag="wv")
            wo_t = w_pool.tile([128, 2, D], BF16, tag="wo")
            # wq then xt[0] first: the very first B1 matmul (Q, e=0) can
            # start before wk and the remaining xt chunks land
            nc.sync.dma_start(wq_t[:], wq_d[:].rearrange("(e p) m -> p e m", p=128))
            xt = []
            t0 = xt_pool.tile([128, N], BF16, tag="xt", name="xt0")
            nc.sync.dma_start(t0[:], xt_d[0:128, :])
            xt.append(t0)
            nc.sync.dma_start(wk_t[:], wk_d[:].rearrange("(e p) m -> p e m", p=128))
            for e in range(1, 8):
                t = xt_pool.tile([128, N], BF16, tag="xt")
                nc.sync.dma_start(t[:], xt_d[e * 128:(e + 1) * 128, :])
                xt.append(t)
            nc.sync.dma_start(wv_t[:], wv_d[:].rearrange("(e p) m -> p e m", p=128))
            nc.sync.dma_start(trineg_t[:], trineg_d[:])
            nc.sync.dma_start(padb_t[:], padb_d[:])

            # V' tile: [keys 128, key-block 16, head 4, 65]; col 64 <- ones
            v4 = big_pool.tile([128, NB, 4, 65], BF16, tag="v4")
            nc.sync.dma_start(
                v4[:, :, :, 64:65],
                ones_d[:].rearrange("p (b h o) -> p b h o", h=4, o=1),
            )
            # wout is not needed until phase D
            nc.sync.dma_start(wo_t[:], wout_d[:].rearrange("(c p) m -> p c m", p=128))

            # Preload the Exp activation table off the critical path: the
            # first real exp otherwise eats a 1.3us ACT_TABLE_LOAD mid-kernel.
            warm_t = work_pool.tile([1, 2], F32, tag="warm", name="warm")
            nc.vector.memset(warm_t[:, 0:1], 1.0)
            nc.scalar.activation(warm_t[:, 1:2], warm_t[:, 0:1], EXP, scale=SCALE)

            qt_pair = [big_pool.tile([128, N], BF16, tag=f"qt{p}", name=f"qt{p}") for p in range(2)]
            kt_pair = [big_pool.tile([128, N], BF16, tag=f"kt{p}", name=f"kt{p}") for p in range(2)]
            ctx_pair = [big_pool.tile([128, N], BF16, tag=f"ctx{p}", name=f"ctx{p}") for p in range(2)]

            # ---- phase B: projections ----
            # e-outer so compute starts when xt[0] lands: per 128-row chunk e
            # all 8 Q/K accumulation tiles (4 q-tiles x {Q,K}) of one head
            # pair advance one step. 8 PSUM banks; weight per (e, Q/K) is
            # loaded once and reused across the 4 q-tile matmuls.
            psb_cm = tc.tile_pool(name="psb", bufs=1, space="PSUM")
            psb = psb_cm.__enter__()
            for pair in range(2):
                ps_q = [
                    psb.tile([128, 512], F32, tag=f"bq{i}", name=f"bq{i}")
                    for i in range(NQT)
                ]
                ps_k = [
                    psb.tile([128, 512], F32, tag=f"bk{i}", name=f"bk{i}")
                    for i in range(NQT)
                ]
                for e in range(8):
                    for w_t, ps in ((wq_t, ps_q), (wk_t, ps_k)):
                        for nq in range(NQT):
                            nc.tensor.matmul(
                                ps[nq][:],
                                w_t[:, e, pair * 128:(pair + 1) * 128],
                                xt[e][:, nq * 512:(nq + 1) * 512],
                                start=(e == 0),
                                stop=(e == 7),
                            )
                # copies split across Scalar (idle in phase B) and DVE so the
                # 8-deep copy chain doesn't serialize on one engine — the
                # next phase's first PSUM writers WAR-wait on these reads
                for nq in range(NQT):
                    nc.scalar.copy(
                        qt_pair[pair][:, nq * 512:(nq + 1) * 512], ps_q[nq][:]
                    )
                    nc.vector.tensor_copy(
                        kt_pair[pair][:, nq * 512:(nq + 1) * 512], ps_k[nq][:]
                    )
            psb_cm.__exit__(None, None, None)

            ps_cm = tc.tile_pool(name="ps_main", bufs=3, space="PSUM")
            ps_main = ps_cm.__enter__()
            psc_cm = tc.tile_pool(name="ps_ctx", bufs=1, space="PSUM")
            ps_ctx = psc_cm.__enter__()
            pt_cm = tc.tile_pool(name="pt", bufs=26)
            pt_pool = pt_cm.__enter__()

            # V natural: [n-block, 4*64] = xT-chunk^T @ WvT-chunk. Emitted
            # lazily inside phase C (blocks land at q-tile starts, exactly
            # where the PE otherwise waits for the exp stream); blocks
            # >= kb_max are fully masked and never computed at all.
            v_next = [0]

            def ensure_v(k):
                while v_next[0] < min(k, kb_max):
                    nb = v_next[0]
                    v_next[0] += 1
                    vps = ps_main.tile(
                        [128, 2, 512], F32, tag="blk", name="vps"
                    )[:, 0, 0:256]
                    for e in range(8):
                        nc.tensor.matmul(
                            vps[:],
                            xt[e][:, nb * 128:(nb + 1) * 128],
                            wv_t[:, e, :],
                            start=(e == 0),
                            stop=(e == 7),
                        )
                    # first blocks copy on Scalar (idle until the first exps)
                    veng = nc.scalar.copy if nb < 4 else nc.vector.tensor_copy
                    veng(
                        v4[:, nb, :, 0:64],
                        vps[:].rearrange("p (h d) -> p h d", h=4),
                    )

            # ---- phase C: attention, head pairs interleaved ----
            # A unit is (head-pair, q-tile). The two heads' S^T matmuls sit
            # at base partitions 0 / 64. PV matmuls run one unit behind their
            # exps so the in-order PE never drains waiting on ScalarE.
            def emit_normalize(pair, hh, qt, ctx_ps):
                hp = slice(64 * hh, 64 * hh + 64)
                # Denominator row copied to partition 0 FIRST (the reciprocal
                # chain is the long pole; also the custom-DVE approx
                # reciprocal mishandles source APs at a nonzero partition
                # offset, so it must read a partition-0 tile).
                dcp = work_pool.tile([1, 512], F32, tag="dcp", name="dcp")
                nc.vector.tensor_copy(dcp[:], ctx_ps[64:65, :])
                rrec = work_pool.tile([1, 512], F32, tag="rrec", name="rrec")
                nc.vector.reciprocal_approx_fast(rrec[:], dcp[:])
                # ctx rows staged to SBUF promptly so the PSUM bank frees for
                # the next unit's PV long before the chain completes
                craw = work_pool.tile([64, 512], F32, tag="craw", name="craw")
                nc.vector.tensor_copy(craw[:], ctx_ps[0:64, :])
                rbr = work_pool.tile([64, 512], F32, tag="rbr", name="rbr")
                nc.gpsimd.partition_broadcast(rbr[:], rrec[:])
                nc.vector.tensor_mul(
                    ctx_pair[pair][hp, qt * 512:(qt + 1) * 512],
                    craw[:],
                    rbr[:],
                )

            def emit_st_exp(pair, qt, nchunks, prev):
                """S^T + mask + exp for both heads, with the previous unit's
                PV matmuls riffled in (they are long-ready and fill the PE
                slots where S^T would stall on the exp pipeline). Returns
                PV descriptors."""
                if prev is None:
                    ppv = []
                else:
                    ppair, pqt, pn, ppv, pctx2 = prev

                def rif(k):
                    # emit previous-unit PV chunks up to index k
                    while ppv and ppv[0][0] <= k:
                        jj, ptt, poff = ppv.pop(0)
                        for hh in range(2):
                            nc.tensor.matmul(
                                pctx2[hh][:, poff:],
                                v4[:, jj, 2 * ppair + hh, :],
                                ptt[:, hh, poff:],
                                start=(jj == 0),
                                stop=(jj == pn - 1),
                                skip_group_check=True,
                            )

                pv = []
                for j in range(nchunks):
                    rif(j)
                    d = j - 4 * qt
                    # exact-causal column trim (keep matmul N >= 256)
                    off = 128 * d if d >= 1 else 0
                    st_ps = ps_main.tile([128, 2, 512], F32, tag="blk", name="blk")
                    for hh in range(2):
                        hp = slice(64 * hh, 64 * hh + 64)
                        nc.tensor.matmul(
                            st_ps[:, hh, off:],
                            kt_pair[pair][hp, j * 128:(j + 1) * 128],
                            qt_pair[pair][hp, qt * 512 + off:(qt + 1) * 512],
                            start=True,
                            stop=True,
                        )
                    if d >= 0:
                        # causal add -30000; with off = 128*d the masked
                        # triangle lies entirely in cols [off, off+128)
                        u0 = 384 - 128 * d + off
                        w = min(128, 512 - off)
                        meng = (
                            nc.gpsimd
                            if os.environ.get("KERNEL_MASK_GPSIMD", "0") == "1"
                            else nc.vector
                        )
                        for hh in range(2):
                            meng.tensor_tensor(
                                st_ps[:, hh, off:off + w],
                                st_ps[:, hh, off:off + w],
                                trineg_t[:, u0:u0 + w],
                                ADD,
                            )
                    pt_t = pt_pool.tile([128, 2, 512], BF16, tag="pt")
                    kw = {}
                    if j >= jpad_min:  # per-key pad bias (same for both heads)
                        kw["bias"] = padb_t[:, j:j + 1]
                    nc.scalar.activation(
                        pt_t[:, :, off:], st_ps[:, :, off:], EXP, scale=SCALE, **kw
                    )
                    pv.append((j, pt_t, off))
                rif(10 ** 9)
                return pv

            def emit_pv(pair, qt, nchunks, pv, ctx2):
                for j, pt_t, off in pv:
                    for hh in range(2):
                        nc.tensor.matmul(
                            ctx2[hh][:, off:],
                            v4[:, j, 2 * pair + hh, :],
                            pt_t[:, hh, off:],
                            start=(j == 0),
                            stop=(j == nchunks - 1),
                            skip_group_check=True,
                        )

            units = [
                (pair, qt, min(4 * qt + 4, kb_max))
                for qt in range(NQT)
                for pair in range(2)
            ]
            done_norms = {q: 0 for q in range(NQT)}
            outproj_pending = []

            def emit_outproj(q):
                # output projection for the 4 n-blocks of q-tile q; one blk
                # tile per nb (fc halves in its two banks), one fp16 staging
                # copy, one DMA.
                F16 = mybir.dt.float16
                for nb in range(4 * q, 4 * q + 4):
                    ps = ps_main.tile([128, 2, 512], F32, tag="blk", name="blk")
                    for fc in range(2):
                        for pr2 in range(2):
                            nc.tensor.matmul(
                                ps[:, fc, :],
                                ctx_pair[pr2][:, nb * 128:(nb + 1) * 128],
                                wo_t[:, pr2, fc * 512:(fc + 1) * 512],
                                start=(pr2 == 0),
                                stop=(pr2 == 1),
                            )
                    # staging casts split across Scalar (fc0) and DVE (fc1)
                    # so neither queue eats the full 1.2us, and the two DMA
                    # halves start as soon as their own cast lands
                    osb = work_pool.tile([128, 2, 512], F16, tag="osb", name="osb")
                    nc.scalar.copy(osb[:, 0, :], ps[:, 0, :])
                    nc.vector.tensor_copy(osb[:, 1, :], ps[:, 1, :])
                    for fc in range(2):
                        nc.sync.dma_start(
                            out_d[nb * 128:(nb + 1) * 128,
                                  fc * 512:(fc + 1) * 512],
                            osb[:, fc, :],
                        )

            def pop_norm():
                npair, nqt, nctx2 = norm_q.pop(0)
                for hh in range(2):
                    emit_normalize(npair, hh, nqt, nctx2[hh])
                done_norms[nqt] += 1
                if done_norms[nqt] == 2:
                    outproj_pending.append(nqt)

            prev_pv = None  # (pair, qt, nchunks, pv_descs, ctx2)
            norm_q = []  # normalize one unit behind the PV
            for pair, qt, nchunks in units:
                # Emit the pending normalize chain FIRST so its DVE/GpSimd
                # work sits ahead of this unit's mask adds in the in-order
                # queues (its data deps completed a unit ago).
                if len(norm_q) > 1:
                    pop_norm()
                # V blocks this unit's PV will need (riffled next iteration)
                ensure_v(nchunks)
                pv = emit_st_exp(pair, qt, nchunks, prev_pv)
                if prev_pv is not None:
                    ppair, pqt, pn, ppv, pctx2 = prev_pv
                    norm_q.append((ppair, pqt, pctx2))
                # Out-projections go AFTER the riffle: by the time the PE
                # reaches them the normalize chain has drained.
                while outproj_pending:
                    emit_outproj(outproj_pending.pop(0))
                ctx2 = [
                    ps_ctx.tile([65, 512], F32, tag=f"ctx{hh}", name=f"ctx{hh}")
                    for hh in range(2)
                ]
                prev_pv = (pair, qt, nchunks, pv, ctx2)
            # Epilogue: the second-to-last unit's normalize chain (its PV
            # finished during the last riffle) is emitted BEFORE the last
            # unit's PV matmuls so the chain drains while the PE works.
            ppair, pqt, pn, ppv, pctx2 = prev_pv
            if norm_q:
                pop_norm()
            emit_pv(ppair, pqt, pn, ppv, pctx2)
            norm_q.append((ppair, pqt, pctx2))
            while norm_q:
                pop_norm()
            while outproj_pending:
                emit_outproj(outproj_pending.pop(0))

            pt_cm.__exit__(None, None, None)
            psc_cm.__exit__(None, None, None)
            ps_cm.__exit__(None, None, None)
            xt_cm.__exit__(None, None, None)

    nc.compile()
    return nc


_PROGRAM_CACHE = {}


def kernel(x, attention_mask, W_Q, W_K, W_V, W_out, b_out):
    global LAST_RESULTS
    from concourse.bass_utils import run_bass_kernel_spmd

    x = np.ascontiguousarray(x, dtype=np.float32)
    attention_mask = np.asarray(attention_mask)
    lengths = attention_mask.astype(np.int64).sum(axis=1)
    kb_max = int(math.ceil(lengths.max() / KBLK))
    jpad_min = int(lengths.min() // KBLK)

    key = (kb_max, jpad_min)
    if key not in _PROGRAM_CACHE:
        _PROGRAM_CACHE[key] = _build_program(kb_max, jpad_min)
    nc = _PROGRAM_CACHE[key]

    # host-side input prep (matmul operands pre-cast to bf16)
    import ml_dtypes
    BF = ml_dtypes.bfloat16
    xT = [np.ascontiguousarray(x[b].T.astype(BF)) for b in range(B)]
    wqT = np.ascontiguousarray(np.asarray(W_Q, dtype=np.float32).T.astype(BF))
    wkT = np.ascontiguousarray(np.asarray(W_K, dtype=np.float32).T.astype(BF))
    wvT = np.ascontiguousarray(np.asarray(W_V, dtype=np.float32).T.astype(BF))
    woT = np.ascontiguousarray(np.asarray(W_out, dtype=np.float32).T.astype(BF))
    # padbias[p, j] = 0 if key j*128+p is real else -3750
    padb = [
        np.ascontiguousarray(
            np.where(attention_mask[b].reshape(16, 128).T != 0, 0.0, NEGB)
        ).astype(np.float32)
        for b in range(B)
    ]
    # trineg[p, u] = NEG if u < p + 384 else 0; slice [384-128d : 896-128d]
    # gives the causal additive mask for a diagonal block with offset 128d.
    pp = np.arange(128)[:, None]
    uu = np.arange(896)[None, :]
    trineg = np.where(uu < pp + 384, NEG, 0.0).astype(np.float32)
    ones65 = np.ones((128, 64), dtype=BF)

    in_maps = []
    for c in range(NCORES):
        b, g = divmod(c, 4)
        sl = slice(g * 256, (g + 1) * 256)
        in_maps.append(
            {
                "xt": xT[b],
                "wq": np.ascontiguousarray(wqT[:, sl]),
                "wk": np.ascontiguousarray(wkT[:, sl]),
                "wv": np.ascontiguousarray(wvT[:, sl]),
                "wout": np.ascontiguousarray(woT[sl, :]),
                "padbias": padb[b],
                "trineg": trineg,
                "ones65": ones65,
            }
        )

    trace = bool(int(os.environ.get("KERNEL_TRACE", "0")))
    ncores_run = int(os.environ.get("KERNEL_NCORES", str(NCORES)))
    res = run_bass_kernel_spmd(
        nc,
        in_maps[:ncores_run],
        core_ids=list(range(ncores_run)),
        trace=trace,
        trace_cores=list(range(ncores_run)) if trace else None,
    )
    LAST_RESULTS = res

    out = np.zeros((B, N, D), dtype=np.float32)
    for c in range(len(res.results)):
        out[c // 4] += res.results[c]["out"].astype(np.float32)
    out += np.asarray(b_out, dtype=np.float32)[None, None, :]
    return out


# revision 45
# speedup vs baseline: 2.2029x; 1.0003x over previous
"""Trainium2 Bass kernel for causal+padded multi-head attention.

Problem: B=2, N=2048, D=1024, H=16 heads (DK=64), fp32 I/O.
  out = softmax(mask(x Wq^T (x Wk^T)^T) / sqrt(DK)) (x Wv^T) Wout^T + b_out

Sharding (8 cores): core c handles batch b=c//4 and heads [4*(c%4), 4*(c%4)+4).
Each core computes a partial output [N, D] (its 4 heads' contribution through
the output projection); the host sums the 4 partials per batch and adds b_out.

On-device layout (per core):
  xT   [1024, 2048]  (host-pretransposed x[b])
  QT/KT stored transposed [dk, n] as head-pair tiles [128, 2048]
  V    stored natural as [128(keys), 16 blocks, 4 heads, 65] with a ones
       column appended (col 64) so P@V' also yields the softmax denominator.
  S^T  computed per (head-pair, q-tile 512, key-block 128) as [128, 2, 512]
       in PSUM: matmul(lhsT=KT slice [64,128], rhs=QT slice [64,512]).
       Causal masking = additive -30000 on PSUM (DVE); padding mask is a
       per-key bias fused into the exp; one exp(0.125*s + bias) on ScalarE
       writes P^T straight to SBUF as bf16.
  ctx'^T [65, 512] accumulated in PSUM over key blocks:
       matmul(lhsT=V' [128,65], rhs=P^T [128,512]); PV matmuls are emitted
       one unit behind their exps so the in-order PE never waits on ScalarE.
  Normalization: r = recip(rowsum) on the [1,512] denominator row via the
       fast approx DVE reciprocal, partition-broadcast to [64,512] (GpSimd),
       one DVE multiply; the whole chain is emitted at the START of the
       following unit so it lands early in the in-order DVE queue, and the
       output projection is emitted after that unit's S^T/PV riffle so the
       PE reaches it only after the chain has drained.
  Out projection: matmul(lhsT=ctxT [128,128], rhs=WoutT [128,512]) acc over
       the two head-pair chunks; PSUM -> SBUF staging copy on DVE.

ScalarE runs ONLY the exps (plus a warm-up activation at kernel start that
preloads the Exp table off the critical path); all copies live on DVE.

Phase B (projections) is restructured e-outer: per 128-row chunk e of xT,
all four Q/K accumulation tiles of a head pair advance one step, so the PE
starts as soon as the first xT chunk + W_Q land instead of after the whole
4.2 MB xT DMA. Q/K for one pair use 8 concurrent PSUM banks.

All matmul operands are bf16 (pre-rounded on host for the inputs; on-device
casts for intermediates); accumulation is fp32 in PSUM, and the softmax /
masking / normalization arithmetic is fp32.
"""

import math
import os

import numpy as np

B, N, D, H = 2, 2048, 1024, 16
DK = D // H  # 64
NCORES = 8
HEADS_PER_CORE = 4
QTILE = 512
KBLK = 128
NEG = -30000.0
NEGB = -3750.0  # pad bias applied after the 0.125 scale inside exp
SCALE = 1.0 / math.sqrt(float(DK))  # 0.125

# Set by run() when tracing is enabled (test.py reads this).
LAST_RESULTS = None


def _build_program(kb_max: int, jpad_min: int):
    import concourse.tile as tile
    from concourse import bacc, mybir

    F32 = mybir.dt.float32
    BF16 = mybir.dt.bfloat16
    EXP = mybir.ActivationFunctionType.Exp
    ADD = mybir.AluOpType.add

    nc = bacc.Bacc(None)

    xt_d = nc.dram_tensor("xt", [D, N], BF16, kind="ExternalInput")
    wq_d = nc.dram_tensor("wq", [D, 256], BF16, kind="ExternalInput")
    wk_d = nc.dram_tensor("wk", [D, 256], BF16, kind="ExternalInput")
    wv_d = nc.dram_tensor("wv", [D, 256], BF16, kind="ExternalInput")
    wout_d = nc.dram_tensor("wout", [256, D], BF16, kind="ExternalInput")
    padb_d = nc.dram_tensor("padbias", [128, 16], F32, kind="ExternalInput")
    trineg_d = nc.dram_tensor("trineg", [128, 896], F32, kind="ExternalInput")
    ones_d = nc.dram_tensor("ones65", [128, 64], BF16, kind="ExternalInput")
    out_d = nc.dram_tensor("out", [N, D], mybir.dt.float16, kind="ExternalOutput")

    NB = N // KBLK  # 16 key/row blocks
    NQT = N // QTILE  # 4 q tiles

    with tile.TileContext(nc) as tc:
        with (
            tc.tile_pool(name="w", bufs=1) as w_pool,
            tc.tile_pool(name="big", bufs=1) as big_pool,
            tc.tile_pool(name="work", bufs=2) as work_pool,
        ):
            # ---- load inputs (order = arrival order; xt right after Q/K
            # weights so phase B starts ~11us in; masks/wout arrive later,
            # ahead of their first use in phase C/D) ----
            xt_cm = tc.tile_pool(name="xt", bufs=8)
            xt_pool = xt_cm.__enter__()
            padb_t = w_pool.tile([128, 16], F32, tag="padb")
            trineg_t = w_pool.tile([128, 896], F32, tag="trineg")
            wq_t = w_pool.tile([128, 8, 256], BF16, tag="wq")
            wk_t = w_pool.tile([128, 8, 256], BF16, tag="wk")
            wv_t = w_pool.tile([128, 8, 256], BF16, tag="wv")
            wo_t = w_pool.tile([128, 2, D], BF16, tag="wo")
            # wq then xt[0] first: the very first B1 matmul (Q, e=0) can
            # start before wk and the remaining xt chunks land
            nc.sync.dma_start(wq_t[:], wq_d[:].rearrange("(e p) m -> p e m", p=128))
            xt = []
            t0 = xt_pool.tile([128, N], BF16, tag="xt", name="xt0")
            nc.sync.dma_start(t0[:], xt_d[0:128, :])
            xt.append(t0)
            nc.sync.dma_start(wk_t[:], wk_d[:].rearrange("(e p) m -> p e m", p=128))
            for e in range(1, 8):
                t = xt_pool.tile([128, N], BF16, tag="xt")
                nc.sync.dma_start(t[:], xt_d[e * 128:(e + 1) * 128, :])
                xt.append(t)
            nc.sync.dma_start(wv_t[:], wv_d[:].rearrange("(e p) m -> p e m", p=128))
            nc.sync.dma_start(trineg_t[:], trineg_d[:])
            nc.sync.dma_start(padb_t[:], padb_d[:])

            # V' tile: [keys 128, key-block 16, head 4, 65]; col 64 <- ones
            v4 = big_pool.tile([128, NB, 4, 65], BF16, tag="v4")
            nc.sync.dma_start(
                v4[:, :, :, 64:65],
                ones_d[:].rearrange("p (b h o) -> p b h o", h=4, o=1),
            )
            # wout is not needed until phase D
            nc.sync.dma_start(wo_t[:], wout_d[:].rearrange("(c p) m -> p c m", p=128))

            # Preload the Exp activation table off the critical path: the
            # first real exp otherwise eats a 1.3us ACT_TABLE_LOAD mid-kernel.
            warm_t = work_pool.tile([1, 2], F32, tag="warm", name="warm")
            nc.vector.memset(warm_t[:, 0:1], 1.0)
            nc.scalar.activation(warm_t[:, 1:2], warm_t[:, 0:1], EXP, scale=SCALE)

            qt_pair = [big_pool.tile([128, N], BF16, tag=f"qt{p}", name=f"qt{p}") for p in range(2)]
            kt_pair = [big_pool.tile([128, N], BF16, tag=f"kt{p}", name=f"kt{p}") for p in range(2)]
            ctx_pair = [big_pool.tile([128, N], BF16, tag=f"ctx{p}", name=f"ctx{p}") for p in range(2)]

            # ---- phase B: projections ----
            # e-outer so compute starts when xt[0] lands: per 128-row chunk e
            # all 8 Q/K accumulation tiles (4 q-tiles x {Q,K}) of one head
            # pair advance one step. 8 PSUM banks; weight per (e, Q/K) is
            # loaded once and reused across the 4 q-tile matmuls.
            psb_cm = tc.tile_pool(name="psb", bufs=1, space="PSUM")
            psb = psb_cm.__enter__()
            for pair in range(2):
                ps_q = [
                    psb.tile([128, 512], F32, tag=f"bq{i}", name=f"bq{i}")
                    for i in range(NQT)
                ]
                ps_k = [
                    psb.tile([128, 512], F32, tag=f"bk{i}", name=f"bk{i}")
                    for i in range(NQT)
                ]
                for e in range(8):
                    for w_t, ps in ((wq_t, ps_q), (wk_t, ps_k)):
                        for nq in range(NQT):
                            nc.tensor.matmul(
                                ps[nq][:],
                                w_t[:, e, pair * 128:(pair + 1) * 128],
                                xt[e][:, nq * 512:(nq + 1) * 512],
                                start=(e == 0),
                                stop=(e == 7),
                            )
                # copies split across Scalar (idle in phase B) and DVE so the
                # 8-deep copy chain doesn't serialize on one engine — the
                # next phase's first PSUM writers WAR-wait on these reads
                for nq in range(NQT):
                    nc.scalar.copy(
                        qt_pair[pair][:, nq * 512:(nq + 1) * 512], ps_q[nq][:]
                    )
                    nc.vector.tensor_copy(
                        kt_pair[pair][:, nq * 512:(nq + 1) * 512], ps_k[nq][:]
                    )
            psb_cm.__exit__(None, None, None)

            ps_cm = tc.tile_pool(name="ps_main", bufs=3, space="PSUM")
            ps_main = ps_cm.__enter__()
            psc_cm = tc.tile_pool(name="ps_ctx", bufs=1, space="PSUM")
            ps_ctx = psc_cm.__enter__()
            pt_cm = tc.tile_pool(name="pt", bufs=26)
            pt_pool = pt_cm.__enter__()

            # V natural: [n-block, 4*64] = xT-chunk^T @ WvT-chunk. Emitted
            # lazily inside phase C (blocks land at q-tile starts, exactly
            # where the PE otherwise waits for the exp stream); blocks
            # >= kb_max are fully masked and never computed at all.
            v_next = [0]

            def ensure_v(k):
                while v_next[0] < min(k, kb_max):
                    nb = v_next[0]
                    v_next[0] += 1
                    vps = ps_main.tile(
                        [128, 2, 512], F32, tag="blk", name="vps"
                    )[:, 0, 0:256]
                    for e in range(8):
                        nc.tensor.matmul(
                            vps[:],
                            xt[e][:, nb * 128:(nb + 1) * 128],
                            wv_t[:, e, :],
                            start=(e == 0),
                            stop=(e == 7),
                        )
                    # first blocks copy on Scalar (idle until the first exps)
                    veng = nc.scalar.copy if nb < 4 else nc.vector.tensor_copy
                    veng(
                        v4[:, nb, :, 0:64],
                        vps[:].rearrange("p (h d) -> p h d", h=4),
                    )

            # ---- phase C: attention, head pairs interleaved ----
            # A unit is (head-pair, q-tile). The two heads' S^T matmuls sit
            # at base partitions 0 / 64. PV matmuls run one unit behind their
            # exps so the in-order PE never drains waiting on ScalarE.
            def emit_normalize_unit(pair, qt, ctx2):
                for hh in range(2):
                    hp = slice(64 * hh, 64 * hh + 64)
                    # Denominator row to partition 0 first (the custom-DVE
                    # approx reciprocal requires a partition-0-based input;
                    # the reciprocal chain is the long pole).
                    dcp = work_pool.tile([1, 512], F32, tag="dcp", name="dcp")
                    nc.vector.tensor_copy(dcp[:], ctx2[hh][64:65, :])
                    rrec = work_pool.tile([1, 512], F32, tag="rrec", name="rrec")
                    nc.vector.reciprocal_approx_fast(rrec[:], dcp[:])
                    # ctx rows staged to SBUF promptly so the PSUM bank
                    # frees for the next unit's PV
                    craw = work_pool.tile([64, 512], F32, tag="craw", name="craw")
                    nc.vector.tensor_copy(craw[:], ctx2[hh][0:64, :])
                    rbr = work_pool.tile([64, 512], F32, tag="rbr", name="rbr")
                    nc.gpsimd.partition_broadcast(rbr[:], rrec[:])
                    nc.vector.tensor_mul(
                        ctx_pair[pair][hp, qt * 512:(qt + 1) * 512],
                        craw[:],
                        rbr[:],
                    )

            def emit_st_exp(pair, qt, nchunks, prev):
                """S^T + mask + exp for both heads, with the previous unit's
                PV matmuls riffled in (they are long-ready and fill the PE
                slots where S^T would stall on the exp pipeline). Returns
                PV descriptors."""
                if prev is None:
                    ppv = []
                else:
                    ppair, pqt, pn, ppv, pctx2 = prev

                def rif(k):
                    # emit previous-unit PV chunks up to index k
                    while ppv and ppv[0][0] <= k:
                        jj, ptt, poff = ppv.pop(0)
                        for hh in range(2):
                            nc.tensor.matmul(
                                pctx2[hh][:, poff:],
                                v4[:, jj, 2 * ppair + hh, :],
                                ptt[:, hh, poff:],
                                start=(jj == 0),
                                stop=(jj == pn - 1),
                                skip_group_check=True,
                            )

                pv = []
                for j in range(nchunks):
                    # PV lags its S^T position by 2 chunks so the exp stream
                    # (which trails the PE) never stalls a riffled PV
                    rif(j - 2)
                    d = j - 4 * qt
                    # exact-causal column trim (keep matmul N >= 256)
                    off = 128 * d if d >= 1 else 0
                    st_ps = ps_main.tile([128, 2, 512], F32, tag="blk", name="blk")
                    for hh in range(2):
                        hp = slice(64 * hh, 64 * hh + 64)
                        nc.tensor.matmul(
                            st_ps[:, hh, off:],
                            kt_pair[pair][hp, j * 128:(j + 1) * 128],
                            qt_pair[pair][hp, qt * 512 + off:(qt + 1) * 512],
                            start=True,
                            stop=True,
                        )
                    if d >= 0:
                        # causal add -30000; with off = 128*d the masked
                        # triangle lies entirely in cols [off, off+128)
                        u0 = 384 - 128 * d + off
                        w = min(128, 512 - off)
                        meng = (
                            nc.gpsimd
                            if os.environ.get("KERNEL_MASK_GPSIMD", "0") == "1"
                            else nc.vector
                        )
                        for hh in range(2):
                            meng.tensor_tensor(
                                st_ps[:, hh, off:off + w],
                                st_ps[:, hh, off:off + w],
                                trineg_t[:, u0:u0 + w],
                                ADD,
                            )
                    pt_t = pt_pool.tile([128, 2, 512], BF16, tag="pt")
                    kw = {}
                    if j >= jpad_min:  # per-key pad bias (same for both heads)
                        kw["bias"] = padb_t[:, j:j + 1]
                    nc.scalar.activation(
                        pt_t[:, :, off:], st_ps[:, :, off:], EXP, scale=SCALE, **kw
                    )
                    pv.append((j, pt_t, off))
                rif(10 ** 9)
                return pv

            def emit_pv(pair, qt, nchunks, pv, ctx2):
                for j, pt_t, off in pv:
                    for hh in range(2):
                        nc.tensor.matmul(
                            ctx2[hh][:, off:],
                            v4[:, j, 2 * pair + hh, :],
                            pt_t[:, hh, off:],
                            start=(j == 0),
                            stop=(j == nchunks - 1),
                            skip_group_check=True,
                        )

            units = [
                (pair, qt, min(4 * qt + 4, kb_max))
                for qt in range(NQT)
                for pair in range(2)
            ]
            done_norms = {q: 0 for q in range(NQT)}
            outproj_pending = []

            def emit_outproj(q):
                # output projection for the 4 n-blocks of q-tile q; one blk
                # tile per nb (fc halves in its two banks), one fp16 staging
                # copy, one DMA.
                F16 = mybir.dt.float16
                for nb in range(4 * q, 4 * q + 4):
                    ps = ps_main.tile([128, 2, 512], F32, tag="blk", name="blk")
                    for fc in range(2):
                        for pr2 in range(2):
                            nc.tensor.matmul(
                                ps[:, fc, :],
                                ctx_pair[pr2][:, nb * 128:(nb + 1) * 128],
                                wo_t[:, pr2, fc * 512:(fc + 1) * 512],
                                start=(pr2 == 0),
                                stop=(pr2 == 1),
                            )
                    # staging casts split across Scalar (fc0) and DVE (fc1)
                    # so neither queue eats the full 1.2us, and the two DMA
                    # halves start as soon as their own cast lands
                    osb = work_pool.tile([128, 2, 512], F16, tag="osb", name="osb")
                    nc.scalar.copy(osb[:, 0, :], ps[:, 0, :])
                    nc.vector.tensor_copy(osb[:, 1, :], ps[:, 1, :])
                    for fc in range(2):
                        nc.sync.dma_start(
                            out_d[nb * 128:(nb + 1) * 128,
                                  fc * 512:(fc + 1) * 512],
                            osb[:, fc, :],
                        )

            def pop_norm():
                npair, nqt, nctx2 = norm_q.pop(0)
                emit_normalize_unit(npair, nqt, nctx2)
                done_norms[nqt] += 1
                if done_norms[nqt] == 2:
                    outproj_pending.append(nqt)

            prev_pv = None  # (pair, qt, nchunks, pv_descs, ctx2)
            norm_q = []  # normalize one unit behind the PV
            for pair, qt, nchunks in units:
                # Emit the pending normalize chain FIRST so its DVE/GpSimd
                # work sits ahead of this unit's mask adds in the in-order
                # queues (its data deps completed a unit ago).
                if len(norm_q) > 1:
                    pop_norm()
                # V blocks this unit's PV will need (riffled next iteration)
                ensure_v(nchunks)
                pv = emit_st_exp(pair, qt, nchunks, prev_pv)
                if prev_pv is not None:
                    ppair, pqt, pn, ppv, pctx2 = prev_pv
                    norm_q.append((ppair, pqt, pctx2))
                # Out-projections go AFTER the riffle: by the time the PE
                # reaches them the normalize chain has drained.
                while outproj_pending:
                    emit_outproj(outproj_pending.pop(0))
                ctx2 = [
                    ps_ctx.tile([65, 512], F32, tag=f"ctx{hh}", name=f"ctx{hh}")
                    for hh in range(2)
                ]
                prev_pv = (pair, qt, nchunks, pv, ctx2)
            # Epilogue: the second-to-last unit's normalize chain (its PV
            # finished during the last riffle) is emitted BEFORE the last
            # unit's PV matmuls so the chain drains while the PE works.
            ppair, pqt, pn, ppv, pctx2 = prev_pv
            if norm_q:
                pop_norm()
            emit_pv(ppair, pqt, pn, ppv, pctx2)
            norm_q.append((ppair, pqt, pctx2))
            while norm_q:
                pop_norm()
            while outproj_pending:
                emit_outproj(outproj_pending.pop(0))

            pt_cm.__exit__(None, None, None)
            psc_cm.__exit__(None, None, None)
            ps_cm.__exit__(None, None, None)
            xt_cm.__exit__(None, None, None)

    nc.compile()
    return nc


_PROGRAM_CACHE = {}


def kernel(x, attention_mask, W_Q, W_K, W_V, W_out, b_out):
    global LAST_RESULTS
    from concourse.bass_utils import run_bass_kernel_spmd

    x = np.ascontiguousarray(x, dtype=np.float32)
    attention_mask = np.asarray(attention_mask)
    lengths = attention_mask.astype(np.int64).sum(axis=1)
    kb_max = int(math.ceil(lengths.max() / KBLK))
    jpad_min = int(lengths.min() // KBLK)

    key = (kb_max, jpad_min)
    if key not in _PROGRAM_CACHE:
        _PROGRAM_CACHE[key] = _build_program(kb_max, jpad_min)
    nc = _PROGRAM_CACHE[key]

    # host-side input prep (matmul operands pre-cast to bf16)
    import ml_dtypes
    BF = ml_dtypes.bfloat16
    xT = [np.ascontiguousarray(x[b].T.astype(BF)) for b in range(B)]
    wqT = np.ascontiguousarray(np.asarray(W_Q, dtype=np.float32).T.astype(BF))
    wkT = np.ascontiguousarray(np.asarray(W_K, dtype=np.float32).T.astype(BF))
    wvT = np.ascontiguousarray(np.asarray(W_V, dtype=np.float32).T.astype(BF))
    woT = np.ascontiguousarray(np.asarray(W_out, dtype=np.float32).T.astype(BF))
    # padbias[p, j] = 0 if key j*128+p is real else -3750
    padb = [
        np.ascontiguousarray(
            np.where(attention_mask[b].reshape(16, 128).T != 0, 0.0, NEGB)
        ).astype(np.float32)
        for b in range(B)
    ]
    # trineg[p, u] = NEG if u < p + 384 else 0; slice [384-128d : 896-128d]
    # gives the causal additive mask for a diagonal block with offset 128d.
    pp = np.arange(128)[:, None]
    uu = np.arange(896)[None, :]
    trineg = np.where(uu < pp + 384, NEG, 0.0).astype(np.float32)
    ones65 = np.ones((128, 64), dtype=BF)

    in_maps = []
    for c in range(NCORES):
        b, g = divmod(c, 4)
        sl = slice(g * 256, (g + 1) * 256)
        in_maps.append(
            {
                "xt": xT[b],
                "wq": np.ascontiguousarray(wqT[:, sl]),
                "wk": np.ascontiguousarray(wkT[:, sl]),
                "wv": np.ascontiguousarray(wvT[:, sl]),
                "wout": np.ascontiguousarray(woT[sl, :]),
                "padbias": padb[b],
                "trineg": trineg,
                "ones65": ones65,
            }
        )

    trace = bool(int(os.environ.get("KERNEL_TRACE", "0")))
    ncores_run = int(os.environ.get("KERNEL_NCORES", str(NCORES)))
    res = run_bass_kernel_spmd(
        nc,
        in_maps[:ncores_run],
        core_ids=list(range(ncores_run)),
        trace=trace,
        trace_cores=list(range(ncores_run)) if trace else None,
    )
    LAST_RESULTS = res

    out = np.zeros((B, N, D), dtype=np.float32)
    for c in range(len(res.results)):
        out[c // 4] += res.results[c]["out"].astype(np.float32)
    out += np.asarray(b_out, dtype=np.float32)[None, None, :]
    return out


# revision 49
# speedup vs baseline: 2.2039x; 1.0004x over previous
"""Trainium2 Bass kernel for causal+padded multi-head attention.

Problem: B=2, N=2048, D=1024, H=16 heads (DK=64), fp32 I/O.
  out = softmax(mask(x Wq^T (x Wk^T)^T) / sqrt(DK)) (x Wv^T) Wout^T + b_out

Sharding (8 cores): core c handles batch b=c//4 and heads [4*(c%4), 4*(c%4)+4).
Each core computes a partial output [N, D] (its 4 heads' contribution through
the output projection); the host sums the 4 partials per batch and adds b_out.

On-device layout (per core):
  xT   [1024, 2048]  (host-pretransposed x[b])
  QT/KT stored transposed [dk, n] as head-pair tiles [128, 2048]
  V    stored natural as [128(keys), 16 blocks, 4 heads, 65] with a ones
       column appended (col 64) so P@V' also yields the softmax denominator.
  S^T  computed per (head-pair, q-tile 512, key-block 128) as [128, 2, 512]
       in PSUM: matmul(lhsT=KT slice [64,128], rhs=QT slice [64,512]).
       Causal masking = additive -30000 on PSUM (DVE); padding mask is a
       per-key bias fused into the exp; one exp(0.125*s + bias) on ScalarE
       writes P^T straight to SBUF as bf16.
  ctx'^T [65, 512] accumulated in PSUM over key blocks:
       matmul(lhsT=V' [128,65], rhs=P^T [128,512]); PV matmuls are emitted
       one unit behind their exps so the in-order PE never waits on ScalarE.
  Normalization: r = recip(rowsum) on the [1,512] denominator row via the
       fast approx DVE reciprocal, partition-broadcast to [64,512] (GpSimd),
       one DVE multiply; the whole chain is emitted at the START of the
       following unit so it lands early in the in-order DVE queue, and the
       output projection is emitted after that unit's S^T/PV riffle so the
       PE reaches it only after the chain has drained.
  Out projection: matmul(lhsT=ctxT [128,128], rhs=WoutT [128,512]) acc over
       the two head-pair chunks; PSUM -> SBUF staging copy on DVE.

ScalarE runs ONLY the exps (plus a warm-up activation at kernel start that
preloads the Exp table off the critical path); all copies live on DVE.

Phase B (projections) is restructured e-outer: per 128-row chunk e of xT,
all four Q/K accumulation tiles of a head pair advance one step, so the PE
starts as soon as the first xT chunk + W_Q land instead of after the whole
4.2 MB xT DMA. Q/K for one pair use 8 concurrent PSUM banks.

All matmul operands are bf16 (pre-rounded on host for the inputs; on-device
casts for intermediates); accumulation is fp32 in PSUM, and the softmax /
masking / normalization arithmetic is fp32.
"""

import math
import os

import numpy as np

B, N, D, H = 2, 2048, 1024, 16
DK = D // H  # 64
NCORES = 8
HEADS_PER_CORE = 4
QTILE = 512
KBLK = 128
NEG = -30000.0
NEGB = -3750.0  # pad bias applied after the 0.125 scale inside exp
SCALE = 1.0 / math.sqrt(float(DK))  # 0.125

# Set by run() when tracing is enabled (test.py reads this).
LAST_RESULTS = None


def _build_program(kb_max: int, jpad_min: int):
    import concourse.tile as tile
    from concourse import bacc, mybir

    F32 = mybir.dt.float32
    BF16 = mybir.dt.bfloat16
    EXP = mybir.ActivationFunctionType.Exp
    ADD = mybir.AluOpType.add

    nc = bacc.Bacc(None)

    xt_d = nc.dram_tensor("xt", [D, N], BF16, kind="ExternalInput")
    wq_d = nc.dram_tensor("wq", [D, 256], BF16, kind="ExternalInput")
    wk_d = nc.dram_tensor("wk", [D, 256], BF16, kind="ExternalInput")
    wv_d = nc.dram_tensor("wv", [D, 256], BF16, kind="ExternalInput")
    wout_d = nc.dram_tensor("wout", [256, D], BF16, kind="ExternalInput")
    padb_d = nc.dram_tensor("padbias", [128, 16], F32, kind="ExternalInput")
    trineg_d = nc.dram_tensor("trineg", [128, 896], F32, kind="ExternalInput")
    ones_d = nc.dram_tensor("ones65", [128, 64], BF16, kind="ExternalInput")
    out_d = nc.dram_tensor("out", [N, D], mybir.dt.float16, kind="ExternalOutput")

    NB = N // KBLK  # 16 key/row blocks
    NQT = N // QTILE  # 4 q tiles

    with tile.TileContext(nc) as tc:
        with (
            tc.tile_pool(name="w", bufs=1) as w_pool,
            tc.tile_pool(name="big", bufs=1) as big_pool,
            tc.tile_pool(name="work", bufs=2) as work_pool,
        ):
            # ---- load inputs (order = arrival order; xt right after Q/K
            # weights so phase B starts ~11us in; masks/wout arrive later,
            # ahead of their first use in phase C/D) ----
            xt_cm = tc.tile_pool(name="xt", bufs=8)
            xt_pool = xt_cm.__enter__()
            padb_t = w_pool.tile([128, 16], F32, tag="padb")
            trineg_t = w_pool.tile([128, 896], F32, tag="trineg")
            wq_t = w_pool.tile([128, 8, 256], BF16, tag="wq")
            wk_t = w_pool.tile([128, 8, 256], BF16, tag="wk")
            wv_t = w_pool.tile([128, 8, 256], BF16, tag="wv")
            wo_t = w_pool.tile([128, 2, D], BF16, tag="wo")
            # wq then xt[0] first: the very first B1 matmul (Q, e=0) can
            # start before wk and the remaining xt chunks land
            nc.sync.dma_start(wq_t[:], wq_d[:].rearrange("(e p) m -> p e m", p=128))
            xt = []
            t0 = xt_pool.tile([128, N], BF16, tag="xt", name="xt0")
            nc.sync.dma_start(t0[:], xt_d[0:128, :])
            xt.append(t0)
            nc.sync.dma_start(wk_t[:], wk_d[:].rearrange("(e p) m -> p e m", p=128))
            for e in range(1, 8):
                t = xt_pool.tile([128, N], BF16, tag="xt")
                nc.sync.dma_start(t[:], xt_d[e * 128:(e + 1) * 128, :])
                xt.append(t)
            nc.sync.dma_start(wv_t[:], wv_d[:].rearrange("(e p) m -> p e m", p=128))
            nc.sync.dma_start(trineg_t[:], trineg_d[:])
            nc.sync.dma_start(padb_t[:], padb_d[:])

            # V' tile: [keys 128, key-block 16, head 4, 65]; col 64 <- ones
            v4 = big_pool.tile([128, NB, 4, 65], BF16, tag="v4")
            nc.sync.dma_start(
                v4[:, :, :, 64:65],
                ones_d[:].rearrange("p (b h o) -> p b h o", h=4, o=1),
            )
            # wout is not needed until phase D
            nc.sync.dma_start(wo_t[:], wout_d[:].rearrange("(c p) m -> p c m", p=128))

            # Preload the Exp activation table off the critical path: the
            # first real exp otherwise eats a 1.3us ACT_TABLE_LOAD mid-kernel.
            warm_t = work_pool.tile([1, 2], F32, tag="warm", name="warm")
            nc.vector.memset(warm_t[:, 0:1], 1.0)
            nc.scalar.activation(warm_t[:, 1:2], warm_t[:, 0:1], EXP, scale=SCALE)

            qt_pair = [big_pool.tile([128, N], BF16, tag=f"qt{p}", name=f"qt{p}") for p in range(2)]
            kt_pair = [big_pool.tile([128, N], BF16, tag=f"kt{p}", name=f"kt{p}") for p in range(2)]
            ctx_pair = [big_pool.tile([128, N], BF16, tag=f"ctx{p}", name=f"ctx{p}") for p in range(2)]

            # ---- phase B: projections ----
            # e-outer so compute starts when xt[0] lands: per 128-row chunk e
            # all 8 Q/K accumulation tiles (4 q-tiles x {Q,K}) of one head
            # pair advance one step. 8 PSUM banks; weight per (e, Q/K) is
            # loaded once and reused across the 4 q-tile matmuls.
            psb_cm = tc.tile_pool(name="psb", bufs=1, space="PSUM")
            psb = psb_cm.__enter__()
            for pair in range(2):
                ps_q = [
                    psb.tile([128, 512], F32, tag=f"bq{i}", name=f"bq{i}")
                    for i in range(NQT)
                ]
                ps_k = [
                    psb.tile([128, 512], F32, tag=f"bk{i}", name=f"bk{i}")
                    for i in range(NQT)
                ]
                for e in range(8):
                    for w_t, ps in ((wq_t, ps_q), (wk_t, ps_k)):
                        for nq in range(NQT):
                            nc.tensor.matmul(
                                ps[nq][:],
                                w_t[:, e, pair * 128:(pair + 1) * 128],
                                xt[e][:, nq * 512:(nq + 1) * 512],
                                start=(e == 0),
                                stop=(e == 7),
                            )
                # copies split across Scalar (idle in phase B) and DVE so the
                # 8-deep copy chain doesn't serialize on one engine — the
                # next phase's first PSUM writers WAR-wait on these reads
                for nq in range(NQT):
                    nc.scalar.copy(
                        qt_pair[pair][:, nq * 512:(nq + 1) * 512], ps_q[nq][:]
                    )
                    nc.vector.tensor_copy(
                        kt_pair[pair][:, nq * 512:(nq + 1) * 512], ps_k[nq][:]
                    )
            psb_cm.__exit__(None, None, None)

            ps_cm = tc.tile_pool(name="ps_main", bufs=3, space="PSUM")
            ps_main = ps_cm.__enter__()
            psc_cm = tc.tile_pool(name="ps_ctx", bufs=1, space="PSUM")
            ps_ctx = psc_cm.__enter__()
            pt_cm = tc.tile_pool(name="pt", bufs=26)
            pt_pool = pt_cm.__enter__()

            # V natural: [n-block, 4*64] = xT-chunk^T @ WvT-chunk. Emitted
            # lazily inside phase C (blocks land at q-tile starts, exactly
            # where the PE otherwise waits for the exp stream); blocks
            # >= kb_max are fully masked and never computed at all.
            v_next = [0]

            def ensure_v(k):
                while v_next[0] < min(k, kb_max):
                    nb = v_next[0]
                    v_next[0] += 1
                    vps = ps_main.tile(
                        [128, 2, 512], F32, tag="blk", name="vps"
                    )[:, 0, 0:256]
                    for e in range(8):
                        nc.tensor.matmul(
                            vps[:],
                            xt[e][:, nb * 128:(nb + 1) * 128],
                            wv_t[:, e, :],
                            start=(e == 0),
                            stop=(e == 7),
                        )
                    # first blocks copy on Scalar (idle until the first exps)
                    veng = nc.scalar.copy if nb < 4 else nc.vector.tensor_copy
                    veng(
                        v4[:, nb, :, 0:64],
                        vps[:].rearrange("p (h d) -> p h d", h=4),
                    )

            # ---- phase C: attention, head pairs interleaved ----
            # A unit is (head-pair, q-tile). The two heads' S^T matmuls sit
            # at base partitions 0 / 64. PV matmuls run one unit behind their
            # exps so the in-order PE never drains waiting on ScalarE.
            def emit_normalize_unit(pair, qt, ctx2):
                for hh in range(2):
                    hp = slice(64 * hh, 64 * hh + 64)
                    # Denominator row to partition 0 first (the custom-DVE
                    # approx reciprocal requires a partition-0-based input;
                    # the reciprocal chain is the long pole).
                    dcp = work_pool.tile([1, 512], F32, tag="dcp", name="dcp")
                    nc.vector.tensor_copy(dcp[:], ctx2[hh][64:65, :])
                    rrec = work_pool.tile([1, 512], F32, tag="rrec", name="rrec")
                    nc.vector.reciprocal_approx_fast(rrec[:], dcp[:])
                    # ctx rows staged to SBUF promptly so the PSUM bank
                    # frees for the next unit's PV
                    craw = work_pool.tile([64, 512], F32, tag="craw", name="craw")
                    nc.vector.tensor_copy(craw[:], ctx2[hh][0:64, :])
                    rbr = work_pool.tile([64, 512], F32, tag="rbr", name="rbr")
                    nc.gpsimd.partition_broadcast(rbr[:], rrec[:])
                    nc.vector.tensor_mul(
                        ctx_pair[pair][hp, qt * 512:(qt + 1) * 512],
                        craw[:],
                        rbr[:],
                    )

            def emit_st_exp(pair, qt, nchunks, prev):
                """S^T + mask + exp for both heads, with the previous unit's
                PV matmuls riffled in (they are long-ready and fill the PE
                slots where S^T would stall on the exp pipeline). Returns
                PV descriptors."""
                if prev is None:
                    ppv = []
                else:
                    ppair, pqt, pn, ppv, pctx2 = prev

                def rif(k):
                    # emit previous-unit PV chunks up to index k
                    while ppv and ppv[0][0] <= k:
                        jj, ptt, poff = ppv.pop(0)
                        for hh in range(2):
                            nc.tensor.matmul(
                                pctx2[hh][:, poff:],
                                v4[:, jj, 2 * ppair + hh, :],
                                ptt[:, hh, poff:],
                                start=(jj == 0),
                                stop=(jj == pn - 1),
                                skip_group_check=True,
                            )

                pv = []
                for j in range(nchunks):
                    # PV lags its S^T position by 2 chunks so the exp stream
                    # (which trails the PE) never stalls a riffled PV
                    rif(j - 2)
                    if op_q and j >= 1:
                        emit_outproj_nb(op_q.pop(0))
                    d = j - 4 * qt
                    # exact-causal column trim (keep matmul N >= 256)
                    off = 128 * d if d >= 1 else 0
                    st_ps = ps_main.tile([128, 2, 512], F32, tag="blk", name="blk")
                    for hh in range(2):
                        hp = slice(64 * hh, 64 * hh + 64)
                        nc.tensor.matmul(
                            st_ps[:, hh, off:],
                            kt_pair[pair][hp, j * 128:(j + 1) * 128],
                            qt_pair[pair][hp, qt * 512 + off:(qt + 1) * 512],
                            start=True,
                            stop=True,
                        )
                    if d >= 0:
                        # causal add -30000; with off = 128*d the masked
                        # triangle lies entirely in cols [off, off+128)
                        u0 = 384 - 128 * d + off
                        w = min(128, 512 - off)
                        meng = (
                            nc.gpsimd
                            if os.environ.get("KERNEL_MASK_GPSIMD", "0") == "1"
                            else nc.vector
                        )
                        for hh in range(2):
                            meng.tensor_tensor(
                                st_ps[:, hh, off:off + w],
                                st_ps[:, hh, off:off + w],
                                trineg_t[:, u0:u0 + w],
                                ADD,
                            )
                    pt_t = pt_pool.tile([128, 2, 512], BF16, tag="pt")
                    kw = {}
                    if j >= jpad_min:  # per-key pad bias (same for both heads)
                        kw["bias"] = padb_t[:, j:j + 1]
                    nc.scalar.activation(
                        pt_t[:, :, off:], st_ps[:, :, off:], EXP, scale=SCALE, **kw
                    )
                    pv.append((j, pt_t, off))
                rif(10 ** 9)
                return pv

            def emit_pv(pair, qt, nchunks, pv, ctx2):
                for j, pt_t, off in pv:
                    for hh in range(2):
                        nc.tensor.matmul(
                            ctx2[hh][:, off:],
                            v4[:, j, 2 * pair + hh, :],
                            pt_t[:, hh, off:],
                            start=(j == 0),
                            stop=(j == nchunks - 1),
                            skip_group_check=True,
                        )

            units = [
                (pair, qt, min(4 * qt + 4, kb_max))
                for qt in range(NQT)
                for pair in range(2)
            ]
            done_norms = {q: 0 for q in range(NQT)}
            outproj_pending = []

            op_q = []  # (nb) blocks of ready out-projections, spread into
            # the next unit's chunk stream one block per chunk

            def emit_outproj_nb(nb):
                # one n-block of the output projection: one blk tile (fc
                # halves in its two banks), fp16 staging casts split across
                # Scalar (fc0) and DVE (fc1), one DMA per half.
                F16 = mybir.dt.float16
                ps = ps_main.tile([128, 2, 512], F32, tag="blk", name="blk")
                for fc in range(2):
                    for pr2 in range(2):
                        nc.tensor.matmul(
                            ps[:, fc, :],
                            ctx_pair[pr2][:, nb * 128:(nb + 1) * 128],
                            wo_t[:, pr2, fc * 512:(fc + 1) * 512],
                            start=(pr2 == 0),
                            stop=(pr2 == 1),
                        )
                osb = work_pool.tile([128, 2, 512], F16, tag="osb", name="osb")
                nc.scalar.copy(osb[:, 0, :], ps[:, 0, :])
                nc.vector.tensor_copy(osb[:, 1, :], ps[:, 1, :])
                for fc in range(2):
                    nc.sync.dma_start(
                        out_d[nb * 128:(nb + 1) * 128,
                              fc * 512:(fc + 1) * 512],
                        osb[:, fc, :],
                    )

            def emit_outproj(q):
                for nb in range(4 * q, 4 * q + 4):
                    emit_outproj_nb(nb)

            def pop_norm():
                npair, nqt, nctx2 = norm_q.pop(0)
                emit_normalize_unit(npair, nqt, nctx2)
                done_norms[nqt] += 1
                if done_norms[nqt] == 2:
                    outproj_pending.append(nqt)

            prev_pv = None  # (pair, qt, nchunks, pv_descs, ctx2)
            norm_q = []  # normalize one unit behind the PV
            for pair, qt, nchunks in units:
                # Emit the pending normalize chain FIRST so its DVE/GpSimd
                # work sits ahead of this unit's mask adds in the in-order
                # queues (its data deps completed a unit ago).
                if len(norm_q) > 1:
                    pop_norm()
                # V blocks this unit's PV will need (riffled next iteration)
                ensure_v(nchunks)
                pv = emit_st_exp(pair, qt, nchunks, prev_pv)
                if prev_pv is not None:
                    ppair, pqt, pn, ppv, pctx2 = prev_pv
                    norm_q.append((ppair, pqt, pctx2))
                # Ready out-projection blocks are queued and spread through
                # the NEXT unit's chunk stream (one per chunk) so each one's
                # normalize dependency has drained by the time the PE gets
                # there.
                while outproj_pending:
                    q = outproj_pending.pop(0)
                    op_q.extend(range(4 * q, 4 * q + 4))
                ctx2 = [
                    ps_ctx.tile([65, 512], F32, tag=f"ctx{hh}", name=f"ctx{hh}")
                    for hh in range(2)
                ]
                prev_pv = (pair, qt, nchunks, pv, ctx2)
            # Epilogue: the second-to-last unit's normalize chain (its PV
            # finished during the last riffle) is emitted BEFORE the last
            # unit's PV matmuls so the chain drains while the PE works.
            ppair, pqt, pn, ppv, pctx2 = prev_pv
            if norm_q:
                pop_norm()
            while op_q:
                emit_outproj_nb(op_q.pop(0))
            emit_pv(ppair, pqt, pn, ppv, pctx2)
            norm_q.append((ppair, pqt, pctx2))
            while norm_q:
                pop_norm()
            while outproj_pending:
                emit_outproj(outproj_pending.pop(0))
            while op_q:
                emit_outproj_nb(op_q.pop(0))

            pt_cm.__exit__(None, None, None)
            psc_cm.__exit__(None, None, None)
            ps_cm.__exit__(None, None, None)
            xt_cm.__exit__(None, None, None)

    nc.compile()
    return nc


_PROGRAM_CACHE = {}


def kernel(x, attention_mask, W_Q, W_K, W_V, W_out, b_out):
    global LAST_RESULTS
    from concourse.bass_utils import run_bass_kernel_spmd

    x = np.ascontiguousarray(x, dtype=np.float32)
    attention_mask = np.asarray(attention_mask)
    lengths = attention_mask.astype(np.int64).sum(axis=1)
    kb_max = int(math.ceil(lengths.max() / KBLK))
    jpad_min = int(lengths.min() // KBLK)

    key = (kb_max, jpad_min)
    if key not in _PROGRAM_CACHE:
        _PROGRAM_CACHE[key] = _build_program(kb_max, jpad_min)
    nc = _PROGRAM_CACHE[key]

    # host-side input prep (matmul operands pre-cast to bf16)
    import ml_dtypes
    BF = ml_dtypes.bfloat16
    xT = [np.ascontiguousarray(x[b].T.astype(BF)) for b in range(B)]
    wqT = np.ascontiguousarray(np.asarray(W_Q, dtype=np.float32).T.astype(BF))
    wkT = np.ascontiguousarray(np.asarray(W_K, dtype=np.float32).T.astype(BF))
    wvT = np.ascontiguousarray(np.asarray(W_V, dtype=np.float32).T.astype(BF))
    woT = np.ascontiguousarray(np.asarray(W_out, dtype=np.float32).T.astype(BF))
    # padbias[p, j] = 0 if key j*128+p is real else -3750
    padb = [
        np.ascontiguousarray(
            np.where(attention_mask[b].reshape(16, 128).T != 0, 0.0, NEGB)
        ).astype(np.float32)
        for b in range(B)
    ]
    # trineg[p, u] = NEG if u < p + 384 else 0; slice [384-128d : 896-128d]
    # gives the causal additive mask for a diagonal block with offset 128d.
    pp = np.arange(128)[:, None]
    uu = np.arange(896)[None, :]
    trineg = np.where(uu < pp + 384, NEG, 0.0).astype(np.float32)
    ones65 = np.ones((128, 64), dtype=BF)

    in_maps = []
    for c in range(NCORES):
        b, g = divmod(c, 4)
        sl = slice(g * 256, (g + 1) * 256)
        in_maps.append(
            {
                "xt": xT[b],
                "wq": np.ascontiguousarray(wqT[:, sl]),
                "wk": np.ascontiguousarray(wkT[:, sl]),
                "wv": np.ascontiguousarray(wvT[:, sl]),
                "wout": np.ascontiguousarray(woT[sl, :]),
                "padbias": padb[b],
                "trineg": trineg,
                "ones65": ones65,
            }
        )

    trace = bool(int(os.environ.get("KERNEL_TRACE", "0")))
    ncores_run = int(os.environ.get("KERNEL_NCORES", str(NCORES)))
    res = run_bass_kernel_spmd(
        nc,
        in_maps[:ncores_run],
        core_ids=list(range(ncores_run)),
        trace=trace,
        trace_cores=list(range(ncores_run)) if trace else None,
    )
    LAST_RESULTS = res

    out = np.zeros((B, N, D), dtype=np.float32)
    for c in range(len(res.results)):
        out[c // 4] += res.results[c]["out"].astype(np.float32)
    out += np.asarray(b_out, dtype=np.float32)[None, None, :]
    return out


# revision 51
# speedup vs baseline: 2.2232x; 1.0088x over previous
"""Trainium2 Bass kernel for causal+padded multi-head attention.

Problem: B=2, N=2048, D=1024, H=16 heads (DK=64), fp32 I/O.
  out = softmax(mask(x Wq^T (x Wk^T)^T) / sqrt(DK)) (x Wv^T) Wout^T + b_out

Sharding (8 cores): core c handles batch b=c//4 and heads [4*(c%4), 4*(c%4)+4).
Each core computes a partial output [N, D] (its 4 heads' contribution through
the output projection); the host sums the 4 partials per batch and adds b_out.

On-device layout (per core):
  xT   [1024, 2048]  (host-pretransposed x[b])
  QT/KT stored transposed [dk, n] as head-pair tiles [128, 2048]
  V    stored natural as [128(keys), 16 blocks, 4 heads, 65] with a ones
       column appended (col 64) so P@V' also yields the softmax denominator.
  S^T  computed per (head-pair, q-tile 512, key-block 128) as [128, 2, 512]
       in PSUM: matmul(lhsT=KT slice [64,128], rhs=QT slice [64,512]).
       Causal masking = additive -30000 on PSUM (DVE); padding mask is a
       per-key bias fused into the exp; one exp(0.125*s + bias) on ScalarE
       writes P^T straight to SBUF as bf16.
  ctx'^T [65, 512] accumulated in PSUM over key blocks:
       matmul(lhsT=V' [128,65], rhs=P^T [128,512]); PV matmuls are emitted
       one unit behind their exps so the in-order PE never waits on ScalarE.
  Normalization: r = recip(rowsum) on the [1,512] denominator row via the
       fast approx DVE reciprocal, partition-broadcast to [64,512] (GpSimd),
       one DVE multiply; the whole chain is emitted at the START of the
       following unit so it lands early in the in-order DVE queue, and the
       output projection is emitted after that unit's S^T/PV riffle so the
       PE reaches it only after the chain has drained.
  Out projection: matmul(lhsT=ctxT [128,128], rhs=WoutT [128,512]) acc over
       the two head-pair chunks; PSUM -> SBUF staging copy on DVE.

ScalarE runs ONLY the exps (plus a warm-up activation at kernel start that
preloads the Exp table off the critical path); all copies live on DVE.

Phase B (projections) is restructured e-outer: per 128-row chunk e of xT,
all four Q/K accumulation tiles of a head pair advance one step, so the PE
starts as soon as the first xT chunk + W_Q land instead of after the whole
4.2 MB xT DMA. Q/K for one pair use 8 concurrent PSUM banks.

All matmul operands are bf16 (pre-rounded on host for the inputs; on-device
casts for intermediates); accumulation is fp32 in PSUM, and the softmax /
masking / normalization arithmetic is fp32.
"""

import math
import os

import numpy as np

B, N, D, H = 2, 2048, 1024, 16
DK = D // H  # 64
NCORES = 8
HEADS_PER_CORE = 4
QTILE = 512
KBLK = 128
NEG = -30000.0
NEGB = -3750.0  # pad bias applied after the 0.125 scale inside exp
SCALE = 1.0 / math.sqrt(float(DK))  # 0.125

# Set by run() when tracing is enabled (test.py reads this).
LAST_RESULTS = None


def _build_program(kb_max: int, jpad_min: int):
    import concourse.tile as tile
    from concourse import bacc, mybir

    F32 = mybir.dt.float32
    BF16 = mybir.dt.bfloat16
    EXP = mybir.ActivationFunctionType.Exp
    ADD = mybir.AluOpType.add

    nc = bacc.Bacc(None)

    xt_d = nc.dram_tensor("xt", [D, N], BF16, kind="ExternalInput")
    wq_d = nc.dram_tensor("wq", [D, 256], BF16, kind="ExternalInput")
    wk_d = nc.dram_tensor("wk", [D, 256], BF16, kind="ExternalInput")
    wv_d = nc.dram_tensor("wv", [D, 256], BF16, kind="ExternalInput")
    wout_d = nc.dram_tensor("wout", [256, D], BF16, kind="ExternalInput")
    padb_d = nc.dram_tensor("padbias", [128, 16], F32, kind="ExternalInput")
    trineg_d = nc.dram_tensor("trineg", [128, 896], F32, kind="ExternalInput")
    ones_d = nc.dram_tensor("ones65", [128, 64], BF16, kind="ExternalInput")
    out_d = nc.dram_tensor("out", [N, D], mybir.dt.float16, kind="ExternalOutput")

    NB = N // KBLK  # 16 key/row blocks
    NQT = N // QTILE  # 4 q tiles

    with tile.TileContext(nc) as tc:
        with (
            tc.tile_pool(name="w", bufs=1) as w_pool,
            tc.tile_pool(name="big", bufs=1) as big_pool,
            tc.tile_pool(name="work", bufs=2) as work_pool,
        ):
            # ---- load inputs (order = arrival order; xt right after Q/K
            # weights so phase B starts ~11us in; masks/wout arrive later,
            # ahead of their first use in phase C/D) ----
            xt_cm = tc.tile_pool(name="xt", bufs=8)
            xt_pool = xt_cm.__enter__()
            padb_t = w_pool.tile([128, 16], F32, tag="padb")
            trineg_t = w_pool.tile([128, 896], F32, tag="trineg")
            wq_t = w_pool.tile([128, 8, 256], BF16, tag="wq")
            wk_t = w_pool.tile([128, 8, 256], BF16, tag="wk")
            wv_t = w_pool.tile([128, 8, 256], BF16, tag="wv")
            wo_t = w_pool.tile([128, 2, D], BF16, tag="wo")
            # wq then xt[0] first: the very first B1 matmul (Q, e=0) can
            # start before wk and the remaining xt chunks land
            nc.sync.dma_start(wq_t[:], wq_d[:].rearrange("(e p) m -> p e m", p=128))
            xt = []
            t0 = xt_pool.tile([128, N], BF16, tag="xt", name="xt0")
            nc.sync.dma_start(t0[:], xt_d[0:128, :])
            xt.append(t0)
            nc.sync.dma_start(wk_t[:], wk_d[:].rearrange("(e p) m -> p e m", p=128))
            for e in range(1, 8):
                t = xt_pool.tile([128, N], BF16, tag="xt")
                nc.sync.dma_start(t[:], xt_d[e * 128:(e + 1) * 128, :])
                xt.append(t)
            nc.sync.dma_start(wv_t[:], wv_d[:].rearrange("(e p) m -> p e m", p=128))
            nc.sync.dma_start(trineg_t[:], trineg_d[:])
            nc.sync.dma_start(padb_t[:], padb_d[:])

            # V' tile: [keys 128, key-block 16, head 4, 65]; col 64 <- ones
            v4 = big_pool.tile([128, NB, 4, 65], BF16, tag="v4")
            nc.sync.dma_start(
                v4[:, :, :, 64:65],
                ones_d[:].rearrange("p (b h o) -> p b h o", h=4, o=1),
            )
            # wout is not needed until phase D
            nc.sync.dma_start(wo_t[:], wout_d[:].rearrange("(c p) m -> p c m", p=128))

            # Preload the Exp activation table off the critical path: the
            # first real exp otherwise eats a 1.3us ACT_TABLE_LOAD mid-kernel.
            warm_t = work_pool.tile([1, 2], F32, tag="warm", name="warm")
            nc.vector.memset(warm_t[:, 0:1], 1.0)
            nc.scalar.activation(warm_t[:, 1:2], warm_t[:, 0:1], EXP, scale=SCALE)

            qt_pair = [big_pool.tile([128, N], BF16, tag=f"qt{p}", name=f"qt{p}") for p in range(2)]
            kt_pair = [big_pool.tile([128, N], BF16, tag=f"kt{p}", name=f"kt{p}") for p in range(2)]
            ctx_pair = [big_pool.tile([128, N], BF16, tag=f"ctx{p}", name=f"ctx{p}") for p in range(2)]

            # ---- phase B: projections ----
            # e-outer so compute starts when xt[0] lands: per 128-row chunk e
            # all 8 Q/K accumulation tiles (4 q-tiles x {Q,K}) of one head
            # pair advance one step. 8 PSUM banks; weight per (e, Q/K) is
            # loaded once and reused across the 4 q-tile matmuls.
            psb_cm = tc.tile_pool(name="psb", bufs=1, space="PSUM")
            psb = psb_cm.__enter__()
            for pair in range(2):
                ps_q = [
                    psb.tile([128, 512], F32, tag=f"bq{i}", name=f"bq{i}")
                    for i in range(NQT)
                ]
                ps_k = [
                    psb.tile([128, 512], F32, tag=f"bk{i}", name=f"bk{i}")
                    for i in range(NQT)
                ]
                for e in range(8):
                    for w_t, ps in ((wq_t, ps_q), (wk_t, ps_k)):
                        for nq in range(NQT):
                            nc.tensor.matmul(
                                ps[nq][:],
                                w_t[:, e, pair * 128:(pair + 1) * 128],
                                xt[e][:, nq * 512:(nq + 1) * 512],
                                start=(e == 0),
                                stop=(e == 7),
                            )
                # copies split across Scalar (idle in phase B) and DVE so the
                # 8-deep copy chain doesn't serialize on one engine — the
                # next phase's first PSUM writers WAR-wait on these reads
                for nq in range(NQT):
                    nc.scalar.copy(
                        qt_pair[pair][:, nq * 512:(nq + 1) * 512], ps_q[nq][:]
                    )
                    nc.vector.tensor_copy(
                        kt_pair[pair][:, nq * 512:(nq + 1) * 512], ps_k[nq][:]
                    )
            psb_cm.__exit__(None, None, None)

            ps_cm = tc.tile_pool(name="ps_main", bufs=3, space="PSUM")
            ps_main = ps_cm.__enter__()
            psc_cm = tc.tile_pool(name="ps_ctx", bufs=1, space="PSUM")
            ps_ctx = psc_cm.__enter__()
            pt_cm = tc.tile_pool(name="pt", bufs=26)
            pt_pool = pt_cm.__enter__()

            # V natural: [n-block, 4*64] = xT-chunk^T @ WvT-chunk. Emitted
            # lazily inside phase C (blocks land at q-tile starts, exactly
            # where the PE otherwise waits for the exp stream); blocks
            # >= kb_max are fully masked and never computed at all.
            v_next = [0]

            def ensure_v(k):
                while v_next[0] < min(k, kb_max):
                    nb = v_next[0]
                    v_next[0] += 1
                    vps = ps_main.tile(
                        [128, 2, 512], F32, tag="blk", name="vps"
                    )[:, 0, 0:256]
                    for e in range(8):
                        nc.tensor.matmul(
                            vps[:],
                            xt[e][:, nb * 128:(nb + 1) * 128],
                            wv_t[:, e, :],
                            start=(e == 0),
                            stop=(e == 7),
                        )
                    # first blocks copy on Scalar (idle until the first exps)
                    veng = nc.scalar.copy if nb < 4 else nc.vector.tensor_copy
                    veng(
                        v4[:, nb, :, 0:64],
                        vps[:].rearrange("p (h d) -> p h d", h=4),
                    )

            # ---- phase C: attention, head pairs interleaved ----
            # A unit is (head-pair, q-tile). The two heads' S^T matmuls sit
            # at base partitions 0 / 64. PV matmuls run one unit behind their
            # exps so the in-order PE never drains waiting on ScalarE.
            def emit_normalize_unit(pair, qt, ctx2):
                for hh in range(2):
                    hp = slice(64 * hh, 64 * hh + 64)
                    # Denominator row to partition 0 first (the custom-DVE
                    # approx reciprocal requires a partition-0-based input;
                    # the reciprocal chain is the long pole).
                    dcp = work_pool.tile([1, 512], F32, tag="dcp", name="dcp")
                    nc.vector.tensor_copy(dcp[:], ctx2[hh][64:65, :])
                    rrec = work_pool.tile([1, 512], F32, tag="rrec", name="rrec")
                    nc.vector.reciprocal_approx_fast(rrec[:], dcp[:])
                    # ctx rows staged to SBUF promptly so the PSUM bank
                    # frees for the next unit's PV
                    craw = work_pool.tile([64, 512], F32, tag="craw", name="craw")
                    nc.vector.tensor_copy(craw[:], ctx2[hh][0:64, :])
                    rbr = work_pool.tile([64, 512], F32, tag="rbr", name="rbr")
                    nc.gpsimd.partition_broadcast(rbr[:], rrec[:])
                    nc.vector.tensor_mul(
                        ctx_pair[pair][hp, qt * 512:(qt + 1) * 512],
                        craw[:],
                        rbr[:],
                    )

            def emit_st_exp(pair, qt, nchunks, prev):
                """S^T + mask + exp for both heads, with the previous unit's
                PV matmuls riffled in (they are long-ready and fill the PE
                slots where S^T would stall on the exp pipeline). Returns
                PV descriptors."""
                if prev is None:
                    ppv = []
                else:
                    ppair, pqt, pn, ppv, pctx2 = prev

                def rif(k):
                    # emit previous-unit PV chunks up to index k
                    while ppv and ppv[0][0] <= k:
                        jj, ptt, poff = ppv.pop(0)
                        for hh in range(2):
                            nc.tensor.matmul(
                                pctx2[hh][:, poff:],
                                v4[:, jj, 2 * ppair + hh, :],
                                ptt[:, hh, poff:],
                                start=(jj == 0),
                                stop=(jj == pn - 1),
                                skip_group_check=True,
                            )

                pv = []
                for j in range(nchunks):
                    # PV lags its S^T position by 2 chunks so the exp stream
                    # (which trails the PE) never stalls a riffled PV
                    rif(j - 2)
                    if op_q and j >= 1:
                        emit_outproj_nb(op_q.pop(0))
                    d = j - 4 * qt
                    # exact-causal column trim (keep matmul N >= 256)
                    off = 128 * d if d >= 1 else 0
                    st_ps = ps_main.tile([128, 2, 512], F32, tag="blk", name="blk")
                    for hh in range(2):
                        hp = slice(64 * hh, 64 * hh + 64)
                        nc.tensor.matmul(
                            st_ps[:, hh, off:],
                            kt_pair[pair][hp, j * 128:(j + 1) * 128],
                            qt_pair[pair][hp, qt * 512 + off:(qt + 1) * 512],
                            start=True,
                            stop=True,
                        )
                    if d >= 0:
                        # causal add -30000; with off = 128*d the masked
                        # triangle lies entirely in cols [off, off+128)
                        u0 = 384 - 128 * d + off
                        w = min(128, 512 - off)
                        meng = (
                            nc.gpsimd
                            if os.environ.get("KERNEL_MASK_GPSIMD", "0") == "1"
                            else nc.vector
                        )
                        for hh in range(2):
                            meng.tensor_tensor(
                                st_ps[:, hh, off:off + w],
                                st_ps[:, hh, off:off + w],
                                trineg_t[:, u0:u0 + w],
                                ADD,
                            )
                    pt_t = pt_pool.tile([128, 2, 512], BF16, tag="pt")
                    kw = {}
                    if j >= jpad_min:  # per-key pad bias (same for both heads)
                        kw["bias"] = padb_t[:, j:j + 1]
                    nc.scalar.activation(
                        pt_t[:, :, off:], st_ps[:, :, off:], EXP, scale=SCALE, **kw
                    )
                    pv.append((j, pt_t, off))
                rif(10 ** 9)
                return pv

            def emit_pv(pair, qt, nchunks, pv, ctx2):
                for j, pt_t, off in pv:
                    for hh in range(2):
                        nc.tensor.matmul(
                            ctx2[hh][:, off:],
                            v4[:, j, 2 * pair + hh, :],
                            pt_t[:, hh, off:],
                            start=(j == 0),
                            stop=(j == nchunks - 1),
                            skip_group_check=True,
                        )

            units = [
                (pair, qt, min(4 * qt + 4, kb_max))
                for qt in range(NQT)
                for pair in range(2)
            ]
            done_norms = {q: 0 for q in range(NQT)}
            outproj_pending = []

            op_q = []  # (nb) blocks of ready out-projections, spread into
            # the next unit's chunk stream one block per chunk

            def emit_outproj_nb(nb):
                # one n-block of the output projection: one blk tile (fc
                # halves in its two banks), fp16 staging casts split across
                # Scalar (fc0) and DVE (fc1), one DMA per half.
                F16 = mybir.dt.float16
                ps = ps_main.tile([128, 2, 512], F32, tag="blk", name="blk")
                for fc in range(2):
                    for pr2 in range(2):
                        nc.tensor.matmul(
                            ps[:, fc, :],
                            ctx_pair[pr2][:, nb * 128:(nb + 1) * 128],
                            wo_t[:, pr2, fc * 512:(fc + 1) * 512],
                            start=(pr2 == 0),
                            stop=(pr2 == 1),
                        )
                osb = work_pool.tile([128, 2, 512], F16, tag="osb", name="osb")
                nc.scalar.copy(osb[:, 0, :], ps[:, 0, :])
                nc.vector.tensor_copy(osb[:, 1, :], ps[:, 1, :])
                for fc in range(2):
                    nc.sync.dma_start(
                        out_d[nb * 128:(nb + 1) * 128,
                              fc * 512:(fc + 1) * 512],
                        osb[:, fc, :],
                    )

            def emit_outproj(q):
                for nb in range(4 * q, 4 * q + 4):
                    emit_outproj_nb(nb)

            def pop_norm():
                npair, nqt, nctx2 = norm_q.pop(0)
                emit_normalize_unit(npair, nqt, nctx2)
                done_norms[nqt] += 1
                if done_norms[nqt] == 2:
                    outproj_pending.append(nqt)

            prev_pv = None  # (pair, qt, nchunks, pv_descs, ctx2)
            norm_q = []  # normalize one unit behind the PV
            for pair, qt, nchunks in units:
                # Emit the pending normalize chain FIRST so its DVE/GpSimd
                # work sits ahead of this unit's mask adds in the in-order
                # queues (its data deps completed a unit ago).
                if len(norm_q) > 1:
                    pop_norm()
                # V blocks this unit's PV will need (riffled next iteration)
                ensure_v(nchunks)
                pv = emit_st_exp(pair, qt, nchunks, prev_pv)
                if prev_pv is not None:
                    ppair, pqt, pn, ppv, pctx2 = prev_pv
                    norm_q.append((ppair, pqt, pctx2))
                # Ready out-projection blocks are queued and spread through
                # the NEXT unit's chunk stream (one per chunk) so each one's
                # normalize dependency has drained by the time the PE gets
                # there.
                while outproj_pending:
                    q = outproj_pending.pop(0)
                    op_q.extend(range(4 * q, 4 * q + 4))
                ctx2 = [
                    ps_ctx.tile([65, 512], F32, tag=f"ctx{hh}", name=f"ctx{hh}")
                    for hh in range(2)
                ]
                prev_pv = (pair, qt, nchunks, pv, ctx2)
            # Epilogue: the second-to-last unit's normalize chain (its PV
            # finished during the last riffle) is emitted BEFORE the last
            # unit's PV matmuls so the chain drains while the PE works.
            ppair, pqt, pn, ppv, pctx2 = prev_pv
            if norm_q:
                pop_norm()
            while op_q:
                emit_outproj_nb(op_q.pop(0))
            emit_pv(ppair, pqt, pn, ppv, pctx2)
            norm_q.append((ppair, pqt, pctx2))
            while norm_q:
                pop_norm()
            while outproj_pending:
                emit_outproj(outproj_pending.pop(0))
            while op_q:
                emit_outproj_nb(op_q.pop(0))

            pt_cm.__exit__(None, None, None)
            psc_cm.__exit__(None, None, None)
            ps_cm.__exit__(None, None, None)
            xt_cm.__exit__(None, None, None)

    nc.compile()
    return nc


_PROGRAM_CACHE = {}


def kernel(x, attention_mask, W_Q, W_K, W_V, W_out, b_out):
    global LAST_RESULTS
    from concourse.bass_utils import run_bass_kernel_spmd

    x = np.ascontiguousarray(x, dtype=np.float32)
    attention_mask = np.asarray(attention_mask)
    lengths = attention_mask.astype(np.int64).sum(axis=1)
    kb_max = int(math.ceil(lengths.max() / KBLK))
    jpad_min = int(lengths.min() // KBLK)

    key = (kb_max, jpad_min)
    if key not in _PROGRAM_CACHE:
        _PROGRAM_CACHE[key] = _build_program(kb_max, jpad_min)
    nc = _PROGRAM_CACHE[key]

    # host-side input prep (matmul operands pre-cast to bf16)
    import ml_dtypes
    BF = ml_dtypes.bfloat16
    xT = [np.ascontiguousarray(x[b].T.astype(BF)) for b in range(B)]
    wqT = np.ascontiguousarray(np.asarray(W_Q, dtype=np.float32).T.astype(BF))
    wkT = np.ascontiguousarray(np.asarray(W_K, dtype=np.float32).T.astype(BF))
    wvT = np.ascontiguousarray(np.asarray(W_V, dtype=np.float32).T.astype(BF))
    woT = np.ascontiguousarray(np.asarray(W_out, dtype=np.float32).T.astype(BF))
    # padbias[p, j] = 0 if key j*128+p is real else -3750
    padb = [
        np.ascontiguousarray(
            np.where(attention_mask[b].reshape(16, 128).T != 0, 0.0, NEGB)
        ).astype(np.float32)
        for b in range(B)
    ]
    # trineg[p, u] = NEG if u < p + 384 else 0; slice [384-128d : 896-128d]
    # gives the causal additive mask for a diagonal block with offset 128d.
    pp = np.arange(128)[:, None]
    uu = np.arange(896)[None, :]
    trineg = np.where(uu < pp + 384, NEG, 0.0).astype(np.float32)
    ones65 = np.ones((128, 64), dtype=BF)

    in_maps = []
    for c in range(NCORES):
        b, g = divmod(c, 4)
        sl = slice(g * 256, (g + 1) * 256)
        in_maps.append(
            {
                "xt": xT[b],
                "wq": np.ascontiguousarray(wqT[:, sl]),
                "wk": np.ascontiguousarray(wkT[:, sl]),
                "wv": np.ascontiguousarray(wvT[:, sl]),
                "wout": np.ascontiguousarray(woT[sl, :]),
                "padbias": padb[b],
                "trineg": trineg,
                "ones65": ones65,
            }
        )

    trace = bool(int(os.environ.get("KERNEL_TRACE", "0")))
    ncores_run = int(os.environ.get("KERNEL_NCORES", str(NCORES)))
    res = run_bass_kernel_spmd(
        nc,
        in_maps[:ncores_run],
        core_ids=list(range(ncores_run)),
        trace=trace,
        trace_cores=list(range(ncores_run)) if trace else None,
    )
    LAST_RESULTS = res

    out = np.zeros((B, N, D), dtype=np.float32)
    for c in range(len(res.results)):
        out[c // 4] += res.results[c]["out"].astype(np.float32)
    out += np.asarray(b_out, dtype=np.float32)[None, None, :]
    return out


# revision 52
# speedup vs baseline: 2.2254x; 1.0010x over previous
"""Trainium2 Bass kernel for causal+padded multi-head attention.

Problem: B=2, N=2048, D=1024, H=16 heads (DK=64), fp32 I/O.
  out = softmax(mask(x Wq^T (x Wk^T)^T) / sqrt(DK)) (x Wv^T) Wout^T + b_out

Sharding (8 cores): core c handles batch b=c//4 and heads [4*(c%4), 4*(c%4)+4).
Each core computes a partial output [N, D] (its 4 heads' contribution through
the output projection); the host sums the 4 partials per batch and adds b_out.

On-device layout (per core):
  xT   [1024, 2048]  (host-pretransposed x[b])
  QT/KT stored transposed [dk, n] as head-pair tiles [128, 2048]
  V    stored natural as [128(keys), 16 blocks, 4 heads, 65] with a ones
       column appended (col 64) so P@V' also yields the softmax denominator.
  S^T  computed per (head-pair, q-tile 512, key-block 128) as [128, 2, 512]
       in PSUM: matmul(lhsT=KT slice [64,128], rhs=QT slice [64,512]).
       Causal masking = additive -30000 on PSUM (DVE); padding mask is a
       per-key bias fused into the exp; one exp(0.125*s + bias) on ScalarE
       writes P^T straight to SBUF as bf16.
  ctx'^T [65, 512] accumulated in PSUM over key blocks:
       matmul(lhsT=V' [128,65], rhs=P^T [128,512]); PV matmuls are emitted
       one unit behind their exps so the in-order PE never waits on ScalarE.
  Normalization: denominator row copied to a partition-0 tile (the custom
       approx-reciprocal DVE op mishandles nonzero partition offsets), 1/x
       via reciprocal_approx_fast (~5x the exact op), partition-broadcast
       to [64,512] (GpSimd), one DVE multiply. The chain is emitted at the
       START of the following unit so it sits early in the in-order DVE
       queue; ctx rows are staged to SBUF right away so the PSUM bank
       frees for the next unit's PV.
  Out projection: per n-block, one [128,2,512] PSUM tile (fc halves in its
       two banks), fp16 staging casts split Scalar/DVE, two DMA halves.
       Ready blocks are spread one-per-chunk through the next unit's
       stream so each block's normalize dependency has drained.

ScalarE runs the exps (the Exp table is preloaded by a warm-up activation
at kernel start) plus the phase-B Q copies and first V copies while idle.

Phase B is e-outer: per 128-row chunk e of xT, all eight Q/K accumulation
tiles of a head pair advance one step across 8 PSUM banks, so the PE
starts as soon as W_Q + the first xT chunk land instead of after the whole
4.2 MB xT DMA (input DMA issue is trigger-rate-bound at ~1.1us/descriptor).
The V projection is emitted lazily inside phase C (blocks land at q-tile
starts, where the PE otherwise waits for the exp stream), and blocks
>= kb_max (fully padded) are never computed.

All matmul operands are bf16 (pre-rounded on host for the inputs; on-device
casts for intermediates); accumulation is fp32 in PSUM, and the softmax /
masking / normalization arithmetic is fp32.
"""

import math
import os

import numpy as np

B, N, D, H = 2, 2048, 1024, 16
DK = D // H  # 64
NCORES = 8
HEADS_PER_CORE = 4
QTILE = 512
KBLK = 128
NEG = -30000.0
NEGB = -3750.0  # pad bias applied after the 0.125 scale inside exp
SCALE = 1.0 / math.sqrt(float(DK))  # 0.125

# Set by run() when tracing is enabled (test.py reads this).
LAST_RESULTS = None


def _build_program(kb_max: int, jpad_min: int):
    import concourse.tile as tile
    from concourse import bacc, mybir

    F32 = mybir.dt.float32
    BF16 = mybir.dt.bfloat16
    EXP = mybir.ActivationFunctionType.Exp
    ADD = mybir.AluOpType.add

    nc = bacc.Bacc(None)

    xt_d = nc.dram_tensor("xt", [D, N], BF16, kind="ExternalInput")
    wq_d = nc.dram_tensor("wq", [D, 256], BF16, kind="ExternalInput")
    wk_d = nc.dram_tensor("wk", [D, 256], BF16, kind="ExternalInput")
    wv_d = nc.dram_tensor("wv", [D, 256], BF16, kind="ExternalInput")
    wout_d = nc.dram_tensor("wout", [256, D], BF16, kind="ExternalInput")
    padb_d = nc.dram_tensor("padbias", [128, 16], F32, kind="ExternalInput")
    trineg_d = nc.dram_tensor("trineg", [128, 896], F32, kind="ExternalInput")
    ones_d = nc.dram_tensor("ones65", [128, 64], BF16, kind="ExternalInput")
    out_d = nc.dram_tensor("out", [N, D], mybir.dt.float16, kind="ExternalOutput")

    NB = N // KBLK  # 16 key/row blocks
    NQT = N // QTILE  # 4 q tiles

    with tile.TileContext(nc) as tc:
        with (
            tc.tile_pool(name="w", bufs=1) as w_pool,
            tc.tile_pool(name="big", bufs=1) as big_pool,
            tc.tile_pool(name="work", bufs=2) as work_pool,
        ):
            # ---- load inputs (order = arrival order; xt right after Q/K
            # weights so phase B starts ~11us in; masks/wout arrive later,
            # ahead of their first use in phase C/D) ----
            xt_cm = tc.tile_pool(name="xt", bufs=8)
            xt_pool = xt_cm.__enter__()
            padb_t = w_pool.tile([128, 16], F32, tag="padb")
            trineg_t = w_pool.tile([128, 896], F32, tag="trineg")
            wq_t = w_pool.tile([128, 8, 256], BF16, tag="wq")
            wk_t = w_pool.tile([128, 8, 256], BF16, tag="wk")
            wv_t = w_pool.tile([128, 8, 256], BF16, tag="wv")
            wo_t = w_pool.tile([128, 2, D], BF16, tag="wo")
            # wq then xt[0] first: the very first B1 matmul (Q, e=0) can
            # start before wk and the remaining xt chunks land
            nc.sync.dma_start(wq_t[:], wq_d[:].rearrange("(e p) m -> p e m", p=128))
            xt = []
            t0 = xt_pool.tile([128, N], BF16, tag="xt", name="xt0")
            nc.sync.dma_start(t0[:], xt_d[0:128, :])
            xt.append(t0)
            nc.sync.dma_start(wk_t[:], wk_d[:].rearrange("(e p) m -> p e m", p=128))
            for e in range(1, 8):
                t = xt_pool.tile([128, N], BF16, tag="xt")
                nc.sync.dma_start(t[:], xt_d[e * 128:(e + 1) * 128, :])
                xt.append(t)
            nc.sync.dma_start(wv_t[:], wv_d[:].rearrange("(e p) m -> p e m", p=128))
            nc.sync.dma_start(trineg_t[:], trineg_d[:])
            nc.sync.dma_start(padb_t[:], padb_d[:])

            # V' tile: [keys 128, key-block 16, head 4, 65]; col 64 <- ones
            v4 = big_pool.tile([128, NB, 4, 65], BF16, tag="v4")
            nc.sync.dma_start(
                v4[:, :, :, 64:65],
                ones_d[:].rearrange("p (b h o) -> p b h o", h=4, o=1),
            )
            # wout is not needed until phase D
            nc.sync.dma_start(wo_t[:], wout_d[:].rearrange("(c p) m -> p c m", p=128))

            # Preload the Exp activation table off the critical path: the
            # first real exp otherwise eats a 1.3us ACT_TABLE_LOAD mid-kernel.
            warm_t = work_pool.tile([1, 2], F32, tag="warm", name="warm")
            nc.vector.memset(warm_t[:, 0:1], 1.0)
            nc.scalar.activation(warm_t[:, 1:2], warm_t[:, 0:1], EXP, scale=SCALE)

            qt_pair = [big_pool.tile([128, N], BF16, tag=f"qt{p}", name=f"qt{p}") for p in range(2)]
            kt_pair = [big_pool.tile([128, N], BF16, tag=f"kt{p}", name=f"kt{p}") for p in range(2)]
            ctx_pair = [big_pool.tile([128, N], BF16, tag=f"ctx{p}", name=f"ctx{p}") for p in range(2)]

            # ---- phase B: projections ----
            # e-outer so compute starts when xt[0] lands: per 128-row chunk e
            # all 8 Q/K accumulation tiles (4 q-tiles x {Q,K}) of one head
            # pair advance one step. 8 PSUM banks; weight per (e, Q/K) is
            # loaded once and reused across the 4 q-tile matmuls.
            psb_cm = tc.tile_pool(name="psb", bufs=1, space="PSUM")
            psb = psb_cm.__enter__()
            for pair in range(2):
                ps_q = [
                    psb.tile([128, 512], F32, tag=f"bq{i}", name=f"bq{i}")
                    for i in range(NQT)
                ]
                ps_k = [
                    psb.tile([128, 512], F32, tag=f"bk{i}", name=f"bk{i}")
                    for i in range(NQT)
                ]
                for e in range(8):
                    for w_t, ps in ((wq_t, ps_q), (wk_t, ps_k)):
                        for nq in range(NQT):
                            nc.tensor.matmul(
                                ps[nq][:],
                                w_t[:, e, pair * 128:(pair + 1) * 128],
                                xt[e][:, nq * 512:(nq + 1) * 512],
                                start=(e == 0),
                                stop=(e == 7),
                            )
                # copies split across Scalar (idle in phase B) and DVE so the
                # 8-deep copy chain doesn't serialize on one engine — the
                # next phase's first PSUM writers WAR-wait on these reads
                for nq in range(NQT):
                    nc.scalar.copy(
                        qt_pair[pair][:, nq * 512:(nq + 1) * 512], ps_q[nq][:]
                    )
                    nc.vector.tensor_copy(
                        kt_pair[pair][:, nq * 512:(nq + 1) * 512], ps_k[nq][:]
                    )
            psb_cm.__exit__(None, None, None)

            ps_cm = tc.tile_pool(name="ps_main", bufs=3, space="PSUM")
            ps_main = ps_cm.__enter__()
            psc_cm = tc.tile_pool(name="ps_ctx", bufs=1, space="PSUM")
            ps_ctx = psc_cm.__enter__()
            pt_cm = tc.tile_pool(name="pt", bufs=26)
            pt_pool = pt_cm.__enter__()

            # V natural: [n-block, 4*64] = xT-chunk^T @ WvT-chunk. Emitted
            # lazily inside phase C (blocks land at q-tile starts, exactly
            # where the PE otherwise waits for the exp stream); blocks
            # >= kb_max are fully masked and never computed at all.
            v_next = [0]

            def ensure_v(k):
                while v_next[0] < min(k, kb_max):
                    nb = v_next[0]
                    v_next[0] += 1
                    vps = ps_main.tile(
                        [128, 2, 512], F32, tag="blk", name="vps"
                    )[:, 0, 0:256]
                    for e in range(8):
                        nc.tensor.matmul(
                            vps[:],
                            xt[e][:, nb * 128:(nb + 1) * 128],
                            wv_t[:, e, :],
                            start=(e == 0),
                            stop=(e == 7),
                        )
                    # first blocks copy on Scalar (idle until the first exps)
                    veng = nc.scalar.copy if nb < 4 else nc.vector.tensor_copy
                    veng(
                        v4[:, nb, :, 0:64],
                        vps[:].rearrange("p (h d) -> p h d", h=4),
                    )

            # ---- phase C: attention, head pairs interleaved ----
            # A unit is (head-pair, q-tile). The two heads' S^T matmuls sit
            # at base partitions 0 / 64. PV matmuls run one unit behind their
            # exps so the in-order PE never drains waiting on ScalarE.
            def emit_normalize_unit(pair, qt, ctx2):
                for hh in range(2):
                    hp = slice(64 * hh, 64 * hh + 64)
                    # Denominator row to partition 0 first (the custom-DVE
                    # approx reciprocal requires a partition-0-based input;
                    # the reciprocal chain is the long pole).
                    dcp = work_pool.tile([1, 512], F32, tag="dcp", name="dcp")
                    nc.vector.tensor_copy(dcp[:], ctx2[hh][64:65, :])
                    rrec = work_pool.tile([1, 512], F32, tag="rrec", name="rrec")
                    nc.vector.reciprocal_approx_fast(rrec[:], dcp[:])
                    # ctx rows staged to SBUF promptly so the PSUM bank
                    # frees for the next unit's PV
                    craw = work_pool.tile([64, 512], F32, tag="craw", name="craw")
                    nc.vector.tensor_copy(craw[:], ctx2[hh][0:64, :])
                    rbr = work_pool.tile([64, 512], F32, tag="rbr", name="rbr")
                    nc.gpsimd.partition_broadcast(rbr[:], rrec[:])
                    nc.vector.tensor_mul(
                        ctx_pair[pair][hp, qt * 512:(qt + 1) * 512],
                        craw[:],
                        rbr[:],
                    )

            def emit_st_exp(pair, qt, nchunks, prev):
                """S^T + mask + exp for both heads, with the previous unit's
                PV matmuls riffled in (they are long-ready and fill the PE
                slots where S^T would stall on the exp pipeline). Returns
                PV descriptors."""
                if prev is None:
                    ppv = []
                else:
                    ppair, pqt, pn, ppv, pctx2 = prev

                def rif(k):
                    # emit previous-unit PV chunks up to index k
                    while ppv and ppv[0][0] <= k:
                        jj, ptt, poff = ppv.pop(0)
                        for hh in range(2):
                            nc.tensor.matmul(
                                pctx2[hh][:, poff:],
                                v4[:, jj, 2 * ppair + hh, :],
                                ptt[:, hh, poff:],
                                start=(jj == 0),
                                stop=(jj == pn - 1),
                                skip_group_check=True,
                            )

                pv = []
                for j in range(nchunks):
                    # PV lags its S^T position by 2 chunks so the exp stream
                    # (which trails the PE) never stalls a riffled PV
                    rif(j - 2)
                    if op_q and j >= 1:
                        emit_outproj_nb(op_q.pop(0))
                    d = j - 4 * qt
                    # exact-causal column trim (keep matmul N >= 256)
                    off = 128 * d if d >= 1 else 0
                    st_ps = ps_main.tile([128, 2, 512], F32, tag="blk", name="blk")
                    for hh in range(2):
                        hp = slice(64 * hh, 64 * hh + 64)
                        nc.tensor.matmul(
                            st_ps[:, hh, off:],
                            kt_pair[pair][hp, j * 128:(j + 1) * 128],
                            qt_pair[pair][hp, qt * 512 + off:(qt + 1) * 512],
                            start=True,
                            stop=True,
                        )
                    if d >= 0:
                        # causal add -30000; with off = 128*d the masked
                        # triangle lies entirely in cols [off, off+128)
                        u0 = 384 - 128 * d + off
                        w = min(128, 512 - off)
                        meng = (
                            nc.gpsimd
                            if os.environ.get("KERNEL_MASK_GPSIMD", "0") == "1"
                            else nc.vector
                        )
                        for hh in range(2):
                            meng.tensor_tensor(
                                st_ps[:, hh, off:off + w],
                                st_ps[:, hh, off:off + w],
                                trineg_t[:, u0:u0 + w],
                                ADD,
                            )
                    pt_t = pt_pool.tile([128, 2, 512], BF16, tag="pt")
                    kw = {}
                    if j >= jpad_min:  # per-key pad bias (same for both heads)
                        kw["bias"] = padb_t[:, j:j + 1]
                    nc.scalar.activation(
                        pt_t[:, :, off:], st_ps[:, :, off:], EXP, scale=SCALE, **kw
                    )
                    pv.append((j, pt_t, off))
                rif(10 ** 9)
                return pv

            def emit_pv(pair, qt, nchunks, pv, ctx2):
                for j, pt_t, off in pv:
                    for hh in range(2):
                        nc.tensor.matmul(
                            ctx2[hh][:, off:],
                            v4[:, j, 2 * pair + hh, :],
                            pt_t[:, hh, off:],
                            start=(j == 0),
                            stop=(j == nchunks - 1),
                            skip_group_check=True,
                        )

            units = [
                (pair, qt, min(4 * qt + 4, kb_max))
                for qt in range(NQT)
                for pair in range(2)
            ]
            done_norms = {q: 0 for q in range(NQT)}
            outproj_pending = []

            op_q = []  # (nb) blocks of ready out-projections, spread into
            # the next unit's chunk stream one block per chunk

            def emit_outproj_nb(nb):
                # one n-block of the output projection: one blk tile (fc
                # halves in its two banks), fp16 staging casts split across
                # Scalar (fc0) and DVE (fc1), one DMA per half.
                F16 = mybir.dt.float16
                ps = ps_main.tile([128, 2, 512], F32, tag="blk", name="blk")
                for fc in range(2):
                    for pr2 in range(2):
                        nc.tensor.matmul(
                            ps[:, fc, :],
                            ctx_pair[pr2][:, nb * 128:(nb + 1) * 128],
                            wo_t[:, pr2, fc * 512:(fc + 1) * 512],
                            start=(pr2 == 0),
                            stop=(pr2 == 1),
                        )
                osb = work_pool.tile([128, 2, 512], F16, tag="osb", name="osb")
                nc.scalar.copy(osb[:, 0, :], ps[:, 0, :])
                nc.vector.tensor_copy(osb[:, 1, :], ps[:, 1, :])
                for fc in range(2):
                    nc.sync.dma_start(
                        out_d[nb * 128:(nb + 1) * 128,
                              fc * 512:(fc + 1) * 512],
                        osb[:, fc, :],
                    )

            def emit_outproj(q):
                for nb in range(4 * q, 4 * q + 4):
                    emit_outproj_nb(nb)

            def pop_norm():
                npair, nqt, nctx2 = norm_q.pop(0)
                emit_normalize_unit(npair, nqt, nctx2)
                done_norms[nqt] += 1
                if done_norms[nqt] == 2:
                    outproj_pending.append(nqt)

            prev_pv = None  # (pair, qt, nchunks, pv_descs, ctx2)
            norm_q = []  # normalize one unit behind the PV
            for pair, qt, nchunks in units:
                # Emit the pending normalize chain FIRST so its DVE/GpSimd
                # work sits ahead of this unit's mask adds in the in-order
                # queues (its data deps completed a unit ago).
                if len(norm_q) > 1:
                    pop_norm()
                # V blocks this unit's PV will need (riffled next iteration)
                ensure_v(nchunks)
                pv = emit_st_exp(pair, qt, nchunks, prev_pv)
                if prev_pv is not None:
                    ppair, pqt, pn, ppv, pctx2 = prev_pv
                    norm_q.append((ppair, pqt, pctx2))
                # Ready out-projection blocks are queued and spread through
                # the NEXT unit's chunk stream (one per chunk) so each one's
                # normalize dependency has drained by the time the PE gets
                # there.
                while outproj_pending:
                    q = outproj_pending.pop(0)
                    op_q.extend(range(4 * q, 4 * q + 4))
                ctx2 = [
                    ps_ctx.tile([65, 512], F32, tag=f"ctx{hh}", name=f"ctx{hh}")
                    for hh in range(2)
                ]
                prev_pv = (pair, qt, nchunks, pv, ctx2)
            # Epilogue: the second-to-last unit's normalize chain (its PV
            # finished during the last riffle) is emitted BEFORE the last
            # unit's PV matmuls so the chain drains while the PE works.
            ppair, pqt, pn, ppv, pctx2 = prev_pv
            if norm_q:
                pop_norm()
            while op_q:
                emit_outproj_nb(op_q.pop(0))
            emit_pv(ppair, pqt, pn, ppv, pctx2)
            norm_q.append((ppair, pqt, pctx2))
            while norm_q:
                pop_norm()
            while outproj_pending:
                emit_outproj(outproj_pending.pop(0))
            while op_q:
                emit_outproj_nb(op_q.pop(0))

            pt_cm.__exit__(None, None, None)
            psc_cm.__exit__(None, None, None)
            ps_cm.__exit__(None, None, None)
            xt_cm.__exit__(None, None, None)

    nc.compile()
    return nc


_PROGRAM_CACHE = {}


def kernel(x, attention_mask, W_Q, W_K, W_V, W_out, b_out):
    global LAST_RESULTS
    from concourse.bass_utils import run_bass_kernel_spmd

    x = np.ascontiguousarray(x, dtype=np.float32)
    attention_mask = np.asarray(attention_mask)
    lengths = attention_mask.astype(np.int64).sum(axis=1)
    kb_max = int(math.ceil(lengths.max() / KBLK))
    jpad_min = int(lengths.min() // KBLK)

    key = (kb_max, jpad_min)
    if key not in _PROGRAM_CACHE:
        _PROGRAM_CACHE[key] = _build_program(kb_max, jpad_min)
    nc = _PROGRAM_CACHE[key]

    # host-side input prep (matmul operands pre-cast to bf16)
    import ml_dtypes
    BF = ml_dtypes.bfloat16
    xT = [np.ascontiguousarray(x[b].T.astype(BF)) for b in range(B)]
    wqT = np.ascontiguousarray(np.asarray(W_Q, dtype=np.float32).T.astype(BF))
    wkT = np.ascontiguousarray(np.asarray(W_K, dtype=np.float32).T.astype(BF))
    wvT = np.ascontiguousarray(np.asarray(W_V, dtype=np.float32).T.astype(BF))
    woT = np.ascontiguousarray(np.asarray(W_out, dtype=np.float32).T.astype(BF))
    # padbias[p, j] = 0 if key j*128+p is real else -3750
    padb = [
        np.ascontiguousarray(
            np.where(attention_mask[b].reshape(16, 128).T != 0, 0.0, NEGB)
        ).astype(np.float32)
        for b in range(B)
    ]
    # trineg[p, u] = NEG if u < p + 384 else 0; slice [384-128d : 896-128d]
    # gives the causal additive mask for a diagonal block with offset 128d.
    pp = np.arange(128)[:, None]
    uu = np.arange(896)[None, :]
    trineg = np.where(uu < pp + 384, NEG, 0.0).astype(np.float32)
    ones65 = np.ones((128, 64), dtype=BF)

    in_maps = []
    for c in range(NCORES):
        b, g = divmod(c, 4)
        sl = slice(g * 256, (g + 1) * 256)
        in_maps.append(
            {
                "xt": xT[b],
                "wq": np.ascontiguousarray(wqT[:, sl]),
                "wk": np.ascontiguousarray(wkT[:, sl]),
                "wv": np.ascontiguousarray(wvT[:, sl]),
                "wout": np.ascontiguousarray(woT[sl, :]),
                "padbias": padb[b],
                "trineg": trineg,
                "ones65": ones65,
            }
        )

    trace = bool(int(os.environ.get("KERNEL_TRACE", "0")))
    ncores_run = int(os.environ.get("KERNEL_NCORES", str(NCORES)))
    res = run_bass_kernel_spmd(
        nc,
        in_maps[:ncores_run],
        core_ids=list(range(ncores_run)),
        trace=trace,
        trace_cores=list(range(ncores_run)) if trace else None,
    )
    LAST_RESULTS = res

    out = np.zeros((B, N, D), dtype=np.float32)
    for c in range(len(res.results)):
        out[c // 4] += res.results[c]["out"].astype(np.float32)
    out += np.asarray(b_out, dtype=np.float32)[None, None, :]
    return out
